# revision 5
# baseline (speedup 1.0000x reference)
"""GATv2 2-layer GNN on 8 Trainium2 NeuronCores (Bass/Tile) — v3.

Strategy (dst-sharded edge parallelism, bf16 PE pipeline):
- Nodes dst-sharded: 6250/core packed into 49 chunks of 128 slots (LPT on edge
  counts, chunks relabeled by descending load so per-chunk tile counts align
  across cores). Per-chunk tile counts are baked into the program.
- Both one-hot matrices (s_T: [slot, edge] for the xi gather, s_en:
  [edge, slot] for the scatter) are precomputed on host and streamed as bf16
  inputs — no on-device one-hot builds.
- Attention logits run on the PE: z is computed TRANSPOSED ([hc, e]) via
  accumulating matmuls (W1-half projection + hck one-hot gather in L1;
  hck gather + per-tile transpose-accumulate of gathered xj in L2), Prelu'd
  on ACT into s_bT, then alpha[e,h] = s_bT_tile.T @ attT_half accumulated
  over the two hc halves. The only per-group DVE op left is the exp-broadcast
  message multiply.
- Layer-1 output is normalized+ReLU'd per chunk, transposed on PE, projected
  through W2 inline, and written bf16 to per-slice buffers; sliced AllGathers
  (pipelined behind the layer-1 chunk loop, small final slice) build the
  replicated h2 table. Layer-2 xj comes from int16 pair-index dma_gathers
  (bf16 rows, 4-deep prefetch over the 4 SWDGE queues).
- Post-MP linears run inline in the layer-2 epilogue.
"""

import numpy as np

N = 50000
E = 800000
IN = 128
HC = 256
H = 4
C64 = 64
OUT = 64
SLOPE = 0.2
NCORES = 8
NPC = N // NCORES
CHUNKS = 49
P = 128
SHARD = CHUNKS * P
GSLOTS = NCORES * SHARD
PAD_DST = 255
NSLICES = 5
GB = 2  # tiles per group

SLICE_SIZES = [12, 12, 12, 10, 3]
SLICE_STARTS = [0, 12, 24, 36, 46]


def _slice_of_chunk(j):
    acc = 0
    for k, s in enumerate(SLICE_SIZES):
        if j < acc + s:
            return k, acc, s
        acc += s
    raise AssertionError


def _pack_core(dst_local, n_nodes=NPC, chunks=CHUNKS):
    """LPT-pack nodes into chunks of <=128, then relabel by load desc."""
    deg = np.bincount(dst_local, minlength=n_nodes)
    order = np.argsort(-deg, kind="stable")
    bin_load = np.zeros(chunks, np.int64)
    bin_cnt = np.zeros(chunks, np.int32)
    bin_members = [[] for _ in range(chunks)]
    for v in order:
        cand = np.where(bin_cnt < P)[0]
        b = cand[np.argmin(bin_load[cand])]
        bin_members[b].append(v)
        bin_load[b] += deg[v]
        bin_cnt[b] += 1
    relabel = np.argsort(-bin_load, kind="stable")
    perm = np.full(chunks * P, -1, np.int64)
    for newb, oldb in enumerate(relabel):
        for k, v in enumerate(bin_members[oldb]):
            perm[newb * P + k] = v
    return perm


def _wrap_idx(flat):
    n = flat.shape[0]
    w = flat.reshape(n // 16, 16).T.astype(np.int16)
    return np.tile(w, (8, 1)).copy()


def prepare(inputs):
    import ml_dtypes
    bf16 = ml_dtypes.bfloat16
    x = np.asarray(inputs["x"], np.float32)
    ei = np.asarray(inputs["edge_index"]).astype(np.int64)
    src, dst = ei[0], ei[1]
    owner = dst // NPC
    dst_local = dst - owner * NPC

    perms = []
    for c in range(NCORES):
        m = owner == c
        perms.append(_pack_core(dst_local[m]))

    # slice-major global slot layout:
    # g(core, chunk j, pos p) = (SLICE_STARTS[k]*NCORES + core*SLICE_SIZES[k]
    #                            + (j - j0)) * 128 + p
    def gslot(core, j, p):
        k, j0, s = _slice_of_chunk(j)
        return (SLICE_STARTS[k] * NCORES + core * s + (j - j0)) * P + p

    pos_of = np.empty(N, np.int64)
    own_row = np.empty((NCORES, SHARD), np.int64)
    for c in range(NCORES):
        perm = perms[c]
        for j in range(CHUNKS):
            for p in range(P):
                v = perm[j * P + p]
                if v >= 0:
                    pos_of[v + c * NPC] = gslot(c, j, p)
                own_row[c, j * P + p] = v

    gsrc = pos_of[src]
    gdst_core = owner
    chunk_of_edge = np.empty(E, np.int64)
    slot_of_edge = np.empty(E, np.int64)
    for c in range(NCORES):
        perm = perms[c]
        loc = np.full(NPC, -1, np.int64)
        valid = perm >= 0
        loc[perm[valid]] = np.nonzero(valid)[0]
        m = owner == c
        lp = loc[dst_local[m]]
        chunk_of_edge[m] = lp // P
        slot_of_edge[m] = lp % P
    par = (gsrc & 1).astype(np.int64)

    ev_lists = [[[] for _ in range(CHUNKS)] for _ in range(NCORES)]
    od_lists = [[[] for _ in range(CHUNKS)] for _ in range(NCORES)]
    for e in range(E):
        tgt = ev_lists if par[e] == 0 else od_lists
        tgt[gdst_core[e]][chunk_of_edge[e]].append(e)

    tcnt_ev = tuple(
        max(1, max((len(ev_lists[c][j]) + P - 1) // P for c in range(NCORES)))
        for j in range(CHUNKS))
    tcnt_od = tuple(
        max(1, max((len(od_lists[c][j]) + P - 1) // P for c in range(NCORES)))
        for j in range(CHUNKS))
    T = [a + b for a, b in zip(tcnt_ev, tcnt_od)]
    off_t = np.concatenate([[0], np.cumsum(T)]).astype(np.int64)
    off_ev = np.concatenate([[0], np.cumsum(tcnt_ev)]).astype(np.int64)
    off_od = np.concatenate([[0], np.cumsum(tcnt_od)]).astype(np.int64)
    TOT_T = int(off_t[-1])
    TOT_EV = int(off_ev[-1])
    TOT_OD = int(off_od[-1])

    xsrcT = np.zeros((NCORES, P, TOT_T * P), bf16)     # [in, tile-major edges]
    ev_idx = np.zeros((NCORES, P, TOT_EV * 8), np.int16)
    od_idx = np.zeros((NCORES, P, TOT_OD * 8), np.int16)
    sT_arr = np.zeros((NCORES, P, TOT_T * P), bf16)    # [slot, edge]
    sen_arr = np.zeros((NCORES, P, TOT_T * P), bf16)   # [edge, slot]

    xT = np.ascontiguousarray(x.T.astype(bf16))  # [IN, N]
    for c in range(NCORES):
        for j in range(CHUNKS):
            for edges, tcnt, toff, idx_arr, ioff in (
                (ev_lists[c][j], tcnt_ev[j], off_t[j], ev_idx, off_ev[j]),
                (od_lists[c][j], tcnt_od[j], off_t[j] + tcnt_ev[j], od_idx,
                 off_od[j]),
            ):
                ne = len(edges)
                earr = np.asarray(edges, np.int64)
                flat = np.zeros(tcnt * P, np.int64)
                if ne:
                    flat[:ne] = gsrc[earr] >> 1
                idx_arr[c, :, ioff * 8:(ioff + tcnt) * 8] = _wrap_idx(flat)
                if ne:
                    cols = toff * P + np.arange(ne)
                    xsrcT[c][:, cols] = xT[:, src[earr]]
                    sl = slot_of_edge[earr]
                    sT_arr[c][sl, cols] = 1.0
                    pp = np.arange(ne) % P
                    sen_arr[c][pp, cols - pp + sl] = 1.0

    x_ownT = np.zeros((NCORES, P, SHARD), bf16)
    for c in range(NCORES):
        valid = own_row[c] >= 0
        x_ownT[c][:, valid] = xT[:, own_row[c][valid] + c * NPC]

    W1T = np.ascontiguousarray(np.asarray(inputs["W1"], np.float32).T).astype(bf16)
    W2T = np.ascontiguousarray(np.asarray(inputs["W2"], np.float32).T).astype(bf16)
    W3T = np.ascontiguousarray(np.asarray(inputs["W3"], np.float32).T).astype(bf16)
    W4T = np.ascontiguousarray(np.asarray(inputs["W4"], np.float32).T).astype(bf16)
    att1 = np.asarray(inputs["att1"], np.float32).reshape(HC)
    att2 = np.asarray(inputs["att2"], np.float32).reshape(HC)

    def att_cols(att_flat):
        # [HC, H] with att_cols[hc, h] = att[hc] iff hc//64 == h
        m = np.zeros((HC, H), np.float32)
        m[np.arange(HC), np.arange(HC) // C64] = att_flat
        return m.astype(bf16)

    att1T = att_cols(att1)
    att2T = att_cols(att2)
    b1 = np.asarray(inputs["b1"], np.float32)
    b2 = np.asarray(inputs["b2"], np.float32)
    b3 = np.asarray(inputs["b3"], np.float32)
    b4 = np.asarray(inputs["b4"], np.float32)
    has_b1 = bool(np.any(b1 != 0.0))
    has_b2 = bool(np.any(b2 != 0.0))
    has_b34 = bool(np.any(b3 != 0.0) or np.any(b4 != 0.0))
    b1_rep = np.tile(b1.reshape(1, HC), (P, 1)).astype(np.float32)

    in_maps = []
    for c in range(NCORES):
        in_maps.append({
            "xsrcT": np.ascontiguousarray(xsrcT[c]),
            "x_ownT": np.ascontiguousarray(x_ownT[c]),
            "ev_idx": np.ascontiguousarray(ev_idx[c]),
            "od_idx": np.ascontiguousarray(od_idx[c]),
            "sT_in": np.ascontiguousarray(sT_arr[c]),
            "sen_in": np.ascontiguousarray(sen_arr[c]),
            "W1T": W1T, "W2T": W2T, "W3T": W3T, "W4T": W4T,
            "att1T": att1T, "att2T": att2T,
            "b1x2_row": (2.0 * b1).reshape(1, HC).astype(bf16),
            "b2_row": b2.reshape(1, HC).astype(bf16),
            "b3_row": b3.reshape(1, OUT).astype(bf16),
            "b4_row": b4.reshape(1, OUT).astype(bf16),
            "b1_rep": b1_rep,
        })
    meta = dict(tcnt_ev=tcnt_ev, tcnt_od=tcnt_od, has_b1=has_b1,
                has_b2=has_b2, has_b34=has_b34)
    return in_maps, perms, meta


# ------------------------------------------------------------- device build

def build(tcnt_ev, tcnt_od, has_b1=False, has_b2=False, has_b34=False):
    import concourse.bacc as bacc
    import concourse.mybir as mybir
    import concourse.tile as tile
    from concourse.masks import make_identity

    dt = mybir.dt
    AF = mybir.ActivationFunctionType
    ALU = mybir.AluOpType

    T = [a + b for a, b in zip(tcnt_ev, tcnt_od)]
    off_t = np.concatenate([[0], np.cumsum(T)]).astype(np.int64)
    off_ev = np.concatenate([[0], np.cumsum(tcnt_ev)]).astype(np.int64)
    off_od = np.concatenate([[0], np.cumsum(tcnt_od)]).astype(np.int64)
    TOT_T = int(off_t[-1])
    TOT_EV = int(off_ev[-1])
    TOT_OD = int(off_od[-1])
    TMAX = max(T)

    nc = bacc.Bacc("TRN2", target_bir_lowering=False, debug=False,
                   num_devices=NCORES, num_swdge_queues=4)

    xsrcT = nc.dram_tensor("xsrcT", [P, TOT_T * P], dt.bfloat16, kind="ExternalInput")
    x_ownT = nc.dram_tensor("x_ownT", [P, SHARD], dt.bfloat16, kind="ExternalInput")
    ev_idx = nc.dram_tensor("ev_idx", [P, TOT_EV * 8], dt.int16, kind="ExternalInput")
    od_idx = nc.dram_tensor("od_idx", [P, TOT_OD * 8], dt.int16, kind="ExternalInput")
    sT_in = nc.dram_tensor("sT_in", [P, TOT_T * P], dt.bfloat16, kind="ExternalInput")
    sen_in = nc.dram_tensor("sen_in", [P, TOT_T * P], dt.bfloat16,
                            kind="ExternalInput")
    W1T = nc.dram_tensor("W1T", [IN, HC], dt.bfloat16, kind="ExternalInput")
    W2T = nc.dram_tensor("W2T", [HC, HC], dt.bfloat16, kind="ExternalInput")
    W3T = nc.dram_tensor("W3T", [HC, OUT], dt.bfloat16, kind="ExternalInput")
    W4T = nc.dram_tensor("W4T", [OUT, OUT], dt.bfloat16, kind="ExternalInput")
    att1T = nc.dram_tensor("att1T", [HC, H], dt.bfloat16, kind="ExternalInput")
    att2T = nc.dram_tensor("att2T", [HC, H], dt.bfloat16, kind="ExternalInput")
    b1x2_row = nc.dram_tensor("b1x2_row", [1, HC], dt.bfloat16, kind="ExternalInput")
    b2_row = nc.dram_tensor("b2_row", [1, HC], dt.bfloat16, kind="ExternalInput")
    b3_row = nc.dram_tensor("b3_row", [1, OUT], dt.bfloat16, kind="ExternalInput")
    b4_row = nc.dram_tensor("b4_row", [1, OUT], dt.bfloat16, kind="ExternalInput")
    b1_rep = nc.dram_tensor("b1_rep", [P, HC], dt.float32, kind="ExternalInput")
    y_shard = nc.dram_tensor("y_shard", [SHARD, OUT], dt.float32, kind="ExternalOutput")

    h2in = [nc.dram_tensor(f"h2in_{k}", [SLICE_SIZES[k] * P, HC], dt.bfloat16)
            for k in range(NSLICES)]
    h2_full = nc.dram_tensor("h2_full", [GSLOTS, HC], dt.bfloat16,
                             addr_space="Shared")
    rg = [list(range(NCORES))]

    with tile.TileContext(nc, num_cores=NCORES) as tc:
        with tc.tile_pool(name="const", bufs=1) as constp:
            identf = constp.tile([P, P], dt.float32)
            make_identity(nc, identf[:])
            ident = constp.tile([P, P], dt.bfloat16)
            nc.scalar.activation(ident[:], identf[:], AF.Copy)
            ones_row = constp.tile([1, P], dt.bfloat16)
            nc.gpsimd.memset(ones_row[:], 1.0)

            att_t = {}
            for l, t_ in ((1, att1T), (2, att2T)):
                at = constp.tile([P, 2, H], dt.bfloat16, name=f"att{l}")
                nc.sync.dma_start(
                    out=at[:], in_=t_[:].rearrange("(a p) h -> p (a h)", a=2))
                att_t[l] = at
            bias_t = {}
            for name, t_, w, need in (("b1x2", b1x2_row, HC, has_b1),
                                      ("b2", b2_row, HC, has_b2),
                                      ("b3", b3_row, OUT, has_b34),
                                      ("b4", b4_row, OUT, has_b34)):
                if need:
                    bt = constp.tile([1, w], dt.bfloat16, name=f"bias_{name}")
                    nc.sync.dma_start(out=bt[:], in_=t_[:])
                    bias_t[name] = bt
            b1r_t = None
            if has_b1:
                b1r_t = constp.tile([P, HC], dt.float32, name="b1rep")
                nc.sync.dma_start(out=b1r_t[:], in_=b1_rep[:])
            wtile = {}
            for name, t_, kk, w in (("w1", W1T, IN, HC),
                                    ("w2lo", W2T[0:P, :], P, HC),
                                    ("w2hi", W2T[P:2 * P, :], P, HC),
                                    ("w3lo", W3T[0:P, :], P, OUT),
                                    ("w3hi", W3T[P:2 * P, :], P, OUT),
                                    ("w4", W4T, OUT, OUT)):
                wt = constp.tile([kk, w], dt.bfloat16, name=f"w_{name}")
                nc.sync.dma_start(out=wt[:], in_=t_ if name not in ("w1", "w4") else t_[:])
                wtile[name] = wt
            xot = constp.tile([P, SHARD], dt.bfloat16, name="xot")
            nc.sync.dma_start(out=xot[:], in_=x_ownT[:])

            def edge_layer(layer):
                att_tile = att_t[layer]
                pairs = h2_full[:].rearrange("(a b) d -> a (b d)", b=2)
                with (
                    tc.tile_pool(name="chio", bufs=3) as chio,
                    tc.tile_pool(name="xin", bufs=4) as xin,
                    tc.tile_pool(name="work", bufs=2) as work,
                    tc.tile_pool(name="gps", bufs=2, space="PSUM") as gps,
                    tc.tile_pool(name="eps", bufs=1, space="PSUM") as eps,
                    tc.tile_pool(name="sps", bufs=1, space="PSUM") as sps,
                ):
                    for j in range(CHUNKS):
                        tj = T[j]
                        tev, tod = tcnt_ev[j], tcnt_od[j]
                        k, j0, sk = _slice_of_chunk(j)

                        # --- per-chunk feature table (projected, +bias) ---
                        if layer == 1:
                            hps = sps.tile([P, HC], dt.float32, tag="pmm")
                            nc.tensor.matmul(out=hps[:],
                                             lhsT=xot[:, j * P:(j + 1) * P],
                                             rhs=wtile["w1"][:],
                                             start=True, stop=not has_b1)
                            if has_b1:
                                nc.tensor.matmul(out=hps[:], lhsT=ones_row[:],
                                                 rhs=bias_t["b1x2"][:],
                                                 start=False, stop=True)
                            hck = work.tile([P, HC], dt.bfloat16, tag="hck")
                            nc.scalar.activation(hck[:], hps[:], AF.Copy)
                        else:
                            hck = work.tile([P, HC], dt.bfloat16, tag="hck")
                            nc.sync.dma_start(
                                out=hck[:],
                                in_=h2in[k][(j - j0) * P:(j - j0 + 1) * P, :])

                        sT = chio.tile([P, TMAX * P], dt.bfloat16, tag="sT")
                        nc.sync.dma_start(
                            out=sT[:, 0:tj * P],
                            in_=sT_in[:, off_t[j] * P:(off_t[j] + tj) * P])
                        sen = chio.tile([P, TMAX * P], dt.bfloat16, tag="sen")
                        nc.sync.dma_start(
                            out=sen[:, 0:tj * P],
                            in_=sen_in[:, off_t[j] * P:(off_t[j] + tj) * P])

                        if layer == 1:
                            xsr = xin.tile([P, TMAX * P], dt.bfloat16, tag="xsr")
                            nc.sync.dma_start(
                                out=xsr[:, 0:tj * P],
                                in_=xsrcT[:, off_t[j] * P:(off_t[j] + tj) * P])
                        else:
                            evi = chio.tile([P, max(tcnt_ev) * 8],
                                            dt.int16, tag="evi")
                            nc.sync.dma_start(
                                out=evi[:, 0:tev * 8],
                                in_=ev_idx[:, off_ev[j] * 8:off_ev[j + 1] * 8])
                            odi = chio.tile([P, max(tcnt_od) * 8],
                                            dt.int16, tag="odi")
                            nc.sync.dma_start(
                                out=odi[:, 0:tod * 8],
                                in_=od_idx[:, off_od[j] * 8:off_od[j + 1] * 8])
                            xj_ev = xin.tile([P, max(tcnt_ev), HC], dt.bfloat16,
                                             tag="xjev")
                            xj_od = xin.tile([P, max(tcnt_od), HC], dt.bfloat16,
                                             tag="xjod")
                            nc.gpsimd.dma_gather(
                                out_ap=xj_ev[:, 0:tev, :], in_ap=pairs[:, 0:HC],
                                idxs_ap=evi[:, 0:tev * 8], num_idxs=tev * P,
                                num_idxs_reg=tev * P, elem_size=HC,
                                elem_step=2 * HC, single_packet=False,
                                queue_num=(2 * j) % 4)
                            nc.gpsimd.dma_gather(
                                out_ap=xj_od[:, 0:tod, :], in_ap=pairs[:, HC:2 * HC],
                                idxs_ap=odi[:, 0:tod * 8], num_idxs=tod * P,
                                num_idxs_reg=tod * P, elem_size=HC,
                                elem_step=2 * HC, single_packet=False,
                                queue_num=(2 * j + 1) % 4)

                        msgden = eps.tile([P, HC + 4], dt.float32, tag="msgden",
                                          bufs=1)
                        groups = []
                        if layer == 1:
                            # no parity constraint: span the whole chunk
                            t0 = 0
                            while t0 < tj:
                                gb = min(GB, tj - t0)
                                groups.append((0, t0, gb, 0))
                                t0 += gb
                        else:
                            for base, tcnt, parity in ((0, tev, 0),
                                                       (tev, tod, 1)):
                                t0 = 0
                                while t0 < tcnt:
                                    gb = min(GB, tcnt - t0)
                                    groups.append((base, t0, gb, parity))
                                    t0 += gb

                        first = True
                        ngroups = len(groups)
                        for gi, (base, t0, gb, parity) in enumerate(groups):
                            glob0 = base + t0  # tile index within chunk

                            # --- z computed transposed: [hc_half, e] ---
                            zt = gps.tile([P, 2, GB * P], dt.float32, tag="zt")
                            for hh in range(2):
                                nc.tensor.matmul(
                                    out=zt[:, hh, 0:gb * P],
                                    lhsT=hck[:, hh * P:(hh + 1) * P],
                                    rhs=sT[:, glob0 * P:(glob0 + gb) * P],
                                    start=True, stop=False)
                            if layer == 1:
                                for hh in range(2):
                                    nc.tensor.matmul(
                                        out=zt[:, hh, 0:gb * P],
                                        lhsT=wtile["w1"][:, hh * P:(hh + 1) * P],
                                        rhs=xsr[:, glob0 * P:(glob0 + gb) * P],
                                        start=False, stop=True)
                                ps_xj = gps.tile([P, GB, HC], dt.float32,
                                                 tag="ps_xj")
                                for i in range(gb):
                                    nc.tensor.matmul(
                                        out=ps_xj[:, i, :],
                                        lhsT=xsr[:, (glob0 + i) * P:
                                                 (glob0 + i + 1) * P],
                                        rhs=wtile["w1"][:],
                                        start=True, stop=True)
                                xj_src = ps_xj
                                xj_off = 0
                            else:
                                pool = xj_ev if parity == 0 else xj_od
                                for hh in range(2):
                                    for i in range(gb):
                                        nc.tensor.matmul(
                                            out=zt[:, hh, i * P:(i + 1) * P],
                                            lhsT=pool[:, t0 + i,
                                                      hh * P:(hh + 1) * P],
                                            rhs=ident[:],
                                            start=False,
                                            stop=(i == gb - 1))
                                xj_src = pool
                                xj_off = t0

                            s_bT = work.tile([P, 2, GB * P], dt.bfloat16,
                                             tag="s_bT")
                            nc.scalar.activation(
                                s_bT[:, :, 0:gb * P].rearrange(
                                    "p a c -> p (a c)") if gb == GB else
                                s_bT[:, 0, 0:gb * P],
                                zt[:, :, 0:gb * P].rearrange(
                                    "p a c -> p (a c)") if gb == GB else
                                zt[:, 0, 0:gb * P],
                                AF.Prelu, alpha=SLOPE)
                            if gb != GB:
                                nc.scalar.activation(
                                    s_bT[:, 1, 0:gb * P], zt[:, 1, 0:gb * P],
                                    AF.Prelu, alpha=SLOPE)

                            # --- alpha[e, h] on PE ---
                            alph = gps.tile([P, GB, H], dt.float32, tag="alph")
                            for i in range(gb):
                                for hh in range(2):
                                    nc.tensor.matmul(
                                        out=alph[:, i, :],
                                        lhsT=s_bT[:, hh, i * P:(i + 1) * P],
                                        rhs=att_tile[:, hh, :],
                                        start=(hh == 0), stop=(hh == 1))

                            msg = work.tile([P, GB, HC + 4], dt.bfloat16,
                                            tag="msg")
                            nc.scalar.activation(
                                msg[:, 0:gb, HC:HC + 4],
                                alph[:, 0:gb, :], AF.Exp)
                            nc.vector.tensor_tensor(
                                out=msg[:, 0:gb, 0:HC].rearrange(
                                    "p a (h c) -> p a h c", h=H),
                                in0=xj_src[:, xj_off:xj_off + gb, :].rearrange(
                                    "p a (h c) -> p a h c", h=H),
                                in1=msg[:, 0:gb, HC:HC + 4].to_broadcast(
                                    [P, gb, H, C64]),
                                op=ALU.mult)
                            for i in range(gb):
                                last = (gi == ngroups - 1) and (i == gb - 1)
                                nc.tensor.matmul(
                                    out=msgden[:],
                                    lhsT=sen[:, (glob0 + i) * P:
                                             (glob0 + i + 1) * P],
                                    rhs=msg[:, i, :], start=first, stop=last)
                                first = False

                        # ---------------- epilogue ----------------
                        den = work.tile([P, H], dt.float32, tag="den")
                        nc.vector.tensor_scalar(
                            out=den[:], in0=msgden[:, HC:HC + 4], scalar1=1e-20,
                            scalar2=None, op0=ALU.max)
                        rden = work.tile([P, H], dt.float32, tag="rden")
                        nc.vector.reciprocal(rden[:], den[:])
                        orl = work.tile([P, HC], dt.bfloat16, tag="orl")
                        if layer == 1 and has_b1:
                            tmp = work.tile([P, HC], dt.float32, tag="tmpb")
                            for h in range(H):
                                nc.vector.scalar_tensor_tensor(
                                    out=tmp[:, h * C64:(h + 1) * C64],
                                    in0=msgden[:, h * C64:(h + 1) * C64],
                                    scalar=rden[:, h:h + 1],
                                    in1=b1r_t[:, h * C64:(h + 1) * C64],
                                    op0=ALU.mult, op1=ALU.add)
                            nc.scalar.activation(orl[:], tmp[:], AF.Relu)
                        else:
                            for h in range(H):
                                nc.scalar.activation(
                                    orl[:, h * C64:(h + 1) * C64],
                                    msgden[:, h * C64:(h + 1) * C64],
                                    AF.Relu, scale=rden[:, h:h + 1])

                        trs = []
                        trpt = sps.tile([P, 2 * P], dt.bfloat16, tag="trp")
                        for half in range(2):
                            trp = trpt[:, half * P:(half + 1) * P]
                            nc.tensor.transpose(
                                out=trp, in_=orl[:, half * P:(half + 1) * P],
                                identity=ident[:])
                            tr = work.tile([P, P], dt.bfloat16, tag=f"trs{half}")
                            nc.scalar.activation(tr[:], trp, AF.Copy)
                            trs.append(tr)

                        if layer == 1:
                            # inline phase B: h2 = relu1 @ W2 + b2
                            h2ps = sps.tile([P, HC], dt.float32, tag="pmm")
                            nc.tensor.matmul(out=h2ps[:], lhsT=trs[0][:],
                                             rhs=wtile["w2lo"][:],
                                             start=True, stop=False)
                            nc.tensor.matmul(out=h2ps[:], lhsT=trs[1][:],
                                             rhs=wtile["w2hi"][:],
                                             start=False, stop=not has_b2)
                            if has_b2:
                                nc.tensor.matmul(out=h2ps[:], lhsT=ones_row[:],
                                                 rhs=bias_t["b2"][:],
                                                 start=False, stop=True)
                            h2b = work.tile([P, HC], dt.bfloat16, tag="h2b")
                            nc.scalar.activation(h2b[:], h2ps[:], AF.Copy)
                            nc.sync.dma_start(
                                out=h2in[k][(j - j0) * P:(j - j0 + 1) * P, :],
                                in_=h2b[:])
                            if j - j0 == sk - 1:
                                nc.gpsimd.collective_compute(
                                    "AllGather", mybir.AluOpType.bypass,
                                    replica_groups=rg,
                                    ins=[h2in[k].ap().opt()],
                                    outs=[h2_full[
                                        SLICE_STARTS[k] * NCORES * P:
                                        (SLICE_STARTS[k] + SLICE_SIZES[k])
                                        * NCORES * P, :].opt()])
                        else:
                            # inline phase C: y = (relu2 @ W3 + b3) @ W4 + b4
                            pmm = sps.tile([P, HC], dt.float32, tag="pmm")
                            ps3 = pmm[:, 0:OUT]
                            nc.tensor.matmul(out=ps3, lhsT=trs[0][:],
                                             rhs=wtile["w3lo"][:],
                                             start=True, stop=False)
                            nc.tensor.matmul(out=ps3, lhsT=trs[1][:],
                                             rhs=wtile["w3hi"][:],
                                             start=False, stop=not has_b34)
                            if has_b34:
                                nc.tensor.matmul(out=ps3, lhsT=ones_row[:],
                                                 rhs=bias_t["b3"][:],
                                                 start=False, stop=True)
                            h3 = work.tile([P, OUT], dt.bfloat16, tag="h3")
                            nc.scalar.activation(h3[:], ps3, AF.Copy)
                            h3tp = trpt[0:OUT, 0:P]
                            nc.tensor.transpose(out=h3tp, in_=h3[:],
                                                identity=ident[:])
                            h3t = work.tile([OUT, P], dt.bfloat16, tag="h3t")
                            nc.scalar.activation(h3t[:], h3tp, AF.Copy)
                            ps4 = pmm[:, OUT:2 * OUT]
                            nc.tensor.matmul(out=ps4, lhsT=h3t[:],
                                             rhs=wtile["w4"][:],
                                             start=True, stop=not has_b34)
                            if has_b34:
                                nc.tensor.matmul(out=ps4, lhsT=ones_row[:],
                                                 rhs=bias_t["b4"][:],
                                                 start=False, stop=True)
                            yt = work.tile([P, OUT], dt.float32, tag="yt")
                            nc.scalar.activation(yt[:], ps4, AF.Copy)
                            nc.sync.dma_start(
                                out=y_shard[j * P:(j + 1) * P, :], in_=yt[:])

            edge_layer(1)
            edge_layer(2)

    nc.compile()
    return nc


# ----------------------------------------------------------------- kernel()

_CACHE = {}


def kernel(**inputs):
    from concourse.bass_utils import run_bass_kernel_spmd

    in_maps, perms, meta = prepare(inputs)
    key = tuple(sorted((k, tuple(v) if isinstance(v, tuple) else v)
                       for k, v in meta.items()))
    if key not in _CACHE:
        _CACHE[key] = build(**meta)
    nc = _CACHE[key]
    res = run_bass_kernel_spmd(nc, in_maps, core_ids=list(range(NCORES)))
    out = np.zeros((N, OUT), np.float32)
    for c in range(NCORES):
        ys = res.results[c]["y_shard"]
        valid = perms[c] >= 0
        out[perms[c][valid] + c * NPC] = ys[valid]
    return out


if __name__ == "__main__":
    import jax
    import reference
    cpu = jax.devices("cpu")[0]
    with jax.default_device(cpu):
        inputs = {k: np.asarray(v) for k, v in reference.setup_inputs().items()}
        exp = np.asarray(reference.reference(**inputs))
    got = kernel(**inputs)
    rel = np.linalg.norm(got - exp) / np.linalg.norm(exp)
    print("Relative error:", rel)


# revision 15
# speedup vs baseline: 1.6290x; 1.6290x over previous
"""GATv2 2-layer GNN on 8 Trainium2 NeuronCores (Bass/Tile) — v3.

Strategy (dst-sharded edge parallelism, bf16 PE pipeline):
- Nodes dst-sharded: 6250/core packed into 49 chunks of 128 slots (LPT on edge
  counts, chunks relabeled by descending load so per-chunk tile counts align
  across cores). Per-chunk tile counts are baked into the program.
- Both one-hot matrices (s_T: [slot, edge] for the xi gather, s_en:
  [edge, slot] for the scatter) are precomputed on host and streamed as bf16
  inputs — no on-device one-hot builds.
- Attention logits run on the PE: z is computed TRANSPOSED ([hc, e]) via
  accumulating matmuls (W1-half projection + hck one-hot gather in L1;
  hck gather + per-tile transpose-accumulate of gathered xj in L2), Prelu'd
  on ACT into s_bT, then alpha[e,h] = s_bT_tile.T @ attT_half accumulated
  over the two hc halves. The only per-group DVE op left is the exp-broadcast
  message multiply.
- Layer-1 output is normalized+ReLU'd per chunk, transposed on PE, projected
  through W2 inline, and written bf16 to per-slice buffers; sliced AllGathers
  (pipelined behind the layer-1 chunk loop, small final slice) build the
  replicated h2 table. Layer-2 xj comes from int16 pair-index dma_gathers
  (bf16 rows, 4-deep prefetch over the 4 SWDGE queues).
- Post-MP linears run inline in the layer-2 epilogue.
"""

import numpy as np

N = 50000
E = 800000
IN = 128
HC = 256
H = 4
C64 = 64
OUT = 64
SLOPE = 0.2
NCORES = 8
NPC = N // NCORES
CHUNKS = 49
P = 128
SHARD = CHUNKS * P
GSLOTS = NCORES * SHARD
PAD_DST = 255
NSLICES = 5
GB = 2  # tiles per group

SLICE_SIZES = [12, 12, 12, 10, 3]
SLICE_STARTS = [0, 12, 24, 36, 46]


def _slice_of_chunk(j):
    acc = 0
    for k, s in enumerate(SLICE_SIZES):
        if j < acc + s:
            return k, acc, s
        acc += s
    raise AssertionError


def _pack_core(dst_local, n_nodes=NPC, chunks=CHUNKS):
    """LPT-pack nodes into chunks of <=128, then relabel by load desc."""
    deg = np.bincount(dst_local, minlength=n_nodes)
    order = np.argsort(-deg, kind="stable")
    bin_load = np.zeros(chunks, np.int64)
    bin_cnt = np.zeros(chunks, np.int32)
    bin_members = [[] for _ in range(chunks)]
    for v in order:
        cand = np.where(bin_cnt < P)[0]
        b = cand[np.argmin(bin_load[cand])]
        bin_members[b].append(v)
        bin_load[b] += deg[v]
        bin_cnt[b] += 1
    relabel = np.argsort(-bin_load, kind="stable")
    perm = np.full(chunks * P, -1, np.int64)
    for newb, oldb in enumerate(relabel):
        for k, v in enumerate(bin_members[oldb]):
            perm[newb * P + k] = v
    return perm


def _wrap_idx(flat):
    n = flat.shape[0]
    w = flat.reshape(n // 16, 16).T.astype(np.int16)
    return np.tile(w, (8, 1)).copy()


def prepare(inputs):
    import ml_dtypes
    bf16 = ml_dtypes.bfloat16
    x = np.asarray(inputs["x"], np.float32)
    ei = np.asarray(inputs["edge_index"]).astype(np.int64)
    src, dst = ei[0], ei[1]
    owner = dst // NPC
    dst_local = dst - owner * NPC

    perms = []
    for c in range(NCORES):
        m = owner == c
        perms.append(_pack_core(dst_local[m]))

    # slice-major global slot layout:
    # g(core, chunk j, pos p) = (SLICE_STARTS[k]*NCORES + core*SLICE_SIZES[k]
    #                            + (j - j0)) * 128 + p
    def gslot(core, j, p):
        k, j0, s = _slice_of_chunk(j)
        return (SLICE_STARTS[k] * NCORES + core * s + (j - j0)) * P + p

    pos_of = np.empty(N, np.int64)
    own_row = np.empty((NCORES, SHARD), np.int64)
    for c in range(NCORES):
        perm = perms[c]
        for j in range(CHUNKS):
            for p in range(P):
                v = perm[j * P + p]
                if v >= 0:
                    pos_of[v + c * NPC] = gslot(c, j, p)
                own_row[c, j * P + p] = v

    gsrc = pos_of[src]
    gdst_core = owner
    chunk_of_edge = np.empty(E, np.int64)
    slot_of_edge = np.empty(E, np.int64)
    for c in range(NCORES):
        perm = perms[c]
        loc = np.full(NPC, -1, np.int64)
        valid = perm >= 0
        loc[perm[valid]] = np.nonzero(valid)[0]
        m = owner == c
        lp = loc[dst_local[m]]
        chunk_of_edge[m] = lp // P
        slot_of_edge[m] = lp % P
    par = (gsrc & 1).astype(np.int64)

    ev_lists = [[[] for _ in range(CHUNKS)] for _ in range(NCORES)]
    od_lists = [[[] for _ in range(CHUNKS)] for _ in range(NCORES)]
    for e in range(E):
        tgt = ev_lists if par[e] == 0 else od_lists
        tgt[gdst_core[e]][chunk_of_edge[e]].append(e)

    tcnt_ev = tuple(
        max(1, max((len(ev_lists[c][j]) + P - 1) // P for c in range(NCORES)))
        for j in range(CHUNKS))
    tcnt_od = tuple(
        max(1, max((len(od_lists[c][j]) + P - 1) // P for c in range(NCORES)))
        for j in range(CHUNKS))
    T = [a + b for a, b in zip(tcnt_ev, tcnt_od)]
    off_t = np.concatenate([[0], np.cumsum(T)]).astype(np.int64)
    off_ev = np.concatenate([[0], np.cumsum(tcnt_ev)]).astype(np.int64)
    off_od = np.concatenate([[0], np.cumsum(tcnt_od)]).astype(np.int64)
    TOT_T = int(off_t[-1])
    TOT_EV = int(off_ev[-1])
    TOT_OD = int(off_od[-1])

    xsrcT = np.zeros((NCORES, P, TOT_T * P), bf16)     # [in, tile-major edges]
    ev_idx = np.zeros((NCORES, P, TOT_EV * 8), np.int16)
    od_idx = np.zeros((NCORES, P, TOT_OD * 8), np.int16)
    sT_arr = np.zeros((NCORES, P, TOT_T * P), bf16)    # [slot, edge]
    sen_arr = np.zeros((NCORES, P, TOT_T * P), bf16)   # [edge, slot]

    xT = np.ascontiguousarray(x.T.astype(bf16))  # [IN, N]
    for c in range(NCORES):
        for j in range(CHUNKS):
            for edges, tcnt, toff, idx_arr, ioff in (
                (ev_lists[c][j], tcnt_ev[j], off_t[j], ev_idx, off_ev[j]),
                (od_lists[c][j], tcnt_od[j], off_t[j] + tcnt_ev[j], od_idx,
                 off_od[j]),
            ):
                ne = len(edges)
                earr = np.asarray(edges, np.int64)
                flat = np.zeros(tcnt * P, np.int64)
                if ne:
                    flat[:ne] = gsrc[earr] >> 1
                idx_arr[c, :, ioff * 8:(ioff + tcnt) * 8] = _wrap_idx(flat)
                if ne:
                    cols = toff * P + np.arange(ne)
                    xsrcT[c][:, cols] = xT[:, src[earr]]
                    sl = slot_of_edge[earr]
                    sT_arr[c][sl, cols] = 1.0
                    pp = np.arange(ne) % P
                    sen_arr[c][pp, cols - pp + sl] = 1.0

    x_ownT = np.zeros((NCORES, P, SHARD), bf16)
    for c in range(NCORES):
        valid = own_row[c] >= 0
        x_ownT[c][:, valid] = xT[:, own_row[c][valid] + c * NPC]

    W1T = np.ascontiguousarray(np.asarray(inputs["W1"], np.float32).T).astype(bf16)
    W2T = np.ascontiguousarray(np.asarray(inputs["W2"], np.float32).T).astype(bf16)
    W3T = np.ascontiguousarray(np.asarray(inputs["W3"], np.float32).T).astype(bf16)
    W4T = np.ascontiguousarray(np.asarray(inputs["W4"], np.float32).T).astype(bf16)
    att1 = np.asarray(inputs["att1"], np.float32).reshape(HC)
    att2 = np.asarray(inputs["att2"], np.float32).reshape(HC)

    def att_cols(att_flat):
        # [P, 2*H]: element (p, a*H+h) = att[a*128+p] iff (a*128+p)//64 == h
        m = np.zeros((HC, H), np.float32)
        m[np.arange(HC), np.arange(HC) // C64] = att_flat
        return np.ascontiguousarray(
            m.reshape(2, P, H).transpose(1, 0, 2).reshape(P, 2 * H)).astype(bf16)

    att1T = att_cols(att1)
    att2T = att_cols(att2)
    b1 = np.asarray(inputs["b1"], np.float32)
    b2 = np.asarray(inputs["b2"], np.float32)
    b3 = np.asarray(inputs["b3"], np.float32)
    b4 = np.asarray(inputs["b4"], np.float32)
    has_b1 = bool(np.any(b1 != 0.0))
    has_b2 = bool(np.any(b2 != 0.0))
    has_b34 = bool(np.any(b3 != 0.0) or np.any(b4 != 0.0))
    b1_rep = np.tile(b1.reshape(1, HC), (P, 1)).astype(np.float32)

    in_maps = []
    for c in range(NCORES):
        in_maps.append({
            "xsrcT": np.ascontiguousarray(xsrcT[c]),
            "x_ownT": np.ascontiguousarray(x_ownT[c]),
            "ev_idx": np.ascontiguousarray(ev_idx[c]),
            "od_idx": np.ascontiguousarray(od_idx[c]),
            "sT_in": np.ascontiguousarray(sT_arr[c]),
            "sen_in": np.ascontiguousarray(sen_arr[c]),
            "W1T": W1T, "W2T": W2T, "W3T": W3T, "W4T": W4T,
            "att1T": att1T, "att2T": att2T,
            "b1x2_row": (2.0 * b1).reshape(1, HC).astype(bf16),
            "b2_row": b2.reshape(1, HC).astype(bf16),
            "b3_row": b3.reshape(1, OUT).astype(bf16),
            "b4_row": b4.reshape(1, OUT).astype(bf16),
            "b1_rep": b1_rep,
        })
    meta = dict(tcnt_ev=tcnt_ev, tcnt_od=tcnt_od, has_b1=has_b1,
                has_b2=has_b2, has_b34=has_b34)
    return in_maps, perms, meta


# ------------------------------------------------------------- device build

def build(tcnt_ev, tcnt_od, has_b1=False, has_b2=False, has_b34=False,
          debug_h2=False):
    import concourse.bacc as bacc
    import concourse.mybir as mybir
    import concourse.tile as tile
    from concourse.masks import make_identity

    dt = mybir.dt
    AF = mybir.ActivationFunctionType
    ALU = mybir.AluOpType

    T = [a + b for a, b in zip(tcnt_ev, tcnt_od)]
    off_t = np.concatenate([[0], np.cumsum(T)]).astype(np.int64)
    off_ev = np.concatenate([[0], np.cumsum(tcnt_ev)]).astype(np.int64)
    off_od = np.concatenate([[0], np.cumsum(tcnt_od)]).astype(np.int64)
    TOT_T = int(off_t[-1])
    TOT_EV = int(off_ev[-1])
    TOT_OD = int(off_od[-1])
    TMAX = max(T)

    nc = bacc.Bacc("TRN2", target_bir_lowering=False, debug=False,
                   num_devices=NCORES, num_swdge_queues=4)

    xsrcT = nc.dram_tensor("xsrcT", [P, TOT_T * P], dt.bfloat16, kind="ExternalInput")
    x_ownT = nc.dram_tensor("x_ownT", [P, SHARD], dt.bfloat16, kind="ExternalInput")
    ev_idx = nc.dram_tensor("ev_idx", [P, TOT_EV * 8], dt.int16, kind="ExternalInput")
    od_idx = nc.dram_tensor("od_idx", [P, TOT_OD * 8], dt.int16, kind="ExternalInput")
    sT_in = nc.dram_tensor("sT_in", [P, TOT_T * P], dt.bfloat16, kind="ExternalInput")
    sen_in = nc.dram_tensor("sen_in", [P, TOT_T * P], dt.bfloat16,
                            kind="ExternalInput")
    W1T = nc.dram_tensor("W1T", [IN, HC], dt.bfloat16, kind="ExternalInput")
    W2T = nc.dram_tensor("W2T", [HC, HC], dt.bfloat16, kind="ExternalInput")
    W3T = nc.dram_tensor("W3T", [HC, OUT], dt.bfloat16, kind="ExternalInput")
    W4T = nc.dram_tensor("W4T", [OUT, OUT], dt.bfloat16, kind="ExternalInput")
    att1T = nc.dram_tensor("att1T", [P, 2 * H], dt.bfloat16, kind="ExternalInput")
    att2T = nc.dram_tensor("att2T", [P, 2 * H], dt.bfloat16, kind="ExternalInput")
    b1x2_row = nc.dram_tensor("b1x2_row", [1, HC], dt.bfloat16, kind="ExternalInput")
    b2_row = nc.dram_tensor("b2_row", [1, HC], dt.bfloat16, kind="ExternalInput")
    b3_row = nc.dram_tensor("b3_row", [1, OUT], dt.bfloat16, kind="ExternalInput")
    b4_row = nc.dram_tensor("b4_row", [1, OUT], dt.bfloat16, kind="ExternalInput")
    b1_rep = nc.dram_tensor("b1_rep", [P, HC], dt.float32, kind="ExternalInput")
    y_shard = nc.dram_tensor("y_shard", [SHARD, OUT], dt.float32, kind="ExternalOutput")
    h2_dbg = (nc.dram_tensor("h2_dbg", [SHARD, HC], dt.bfloat16,
                             kind="ExternalOutput") if debug_h2 else None)

    h2in = [nc.dram_tensor(f"h2in_{k}", [SLICE_SIZES[k] * P, HC], dt.bfloat16)
            for k in range(NSLICES)]
    h2_full = nc.dram_tensor("h2_full", [GSLOTS, HC], dt.bfloat16,
                             addr_space="Shared")
    rg = [list(range(NCORES))]

    with tile.TileContext(nc, num_cores=NCORES) as tc:
        with tc.tile_pool(name="const", bufs=1) as constp:
            identf = constp.tile([P, P], dt.float32)
            make_identity(nc, identf[:])
            ident = constp.tile([P, P], dt.bfloat16)
            nc.scalar.activation(ident[:], identf[:], AF.Copy)
            ones_row = constp.tile([1, P], dt.bfloat16)
            nc.gpsimd.memset(ones_row[:], 1.0)

            att_t = {}
            for l, t_ in ((1, att1T), (2, att2T)):
                at = constp.tile([P, 2, H], dt.bfloat16, name=f"att{l}")
                nc.sync.dma_start(out=at[:], in_=t_[:])
                att_t[l] = at
            bias_t = {}
            for name, t_, w, need in (("b1x2", b1x2_row, HC, has_b1),
                                      ("b2", b2_row, HC, has_b2),
                                      ("b3", b3_row, OUT, has_b34),
                                      ("b4", b4_row, OUT, has_b34)):
                if need:
                    bt = constp.tile([1, w], dt.bfloat16, name=f"bias_{name}")
                    nc.sync.dma_start(out=bt[:], in_=t_[:])
                    bias_t[name] = bt
            b1r_t = None
            if has_b1:
                b1r_t = constp.tile([P, HC], dt.float32, name="b1rep")
                nc.sync.dma_start(out=b1r_t[:], in_=b1_rep[:])
            wtile = {}
            for name, t_, kk, w in (("w1", W1T, IN, HC),
                                    ("w2lo", W2T[0:P, :], P, HC),
                                    ("w2hi", W2T[P:2 * P, :], P, HC),
                                    ("w3lo", W3T[0:P, :], P, OUT),
                                    ("w3hi", W3T[P:2 * P, :], P, OUT),
                                    ("w4", W4T, OUT, OUT)):
                wt = constp.tile([kk, w], dt.bfloat16, name=f"w_{name}")
                nc.sync.dma_start(out=wt[:], in_=t_ if name not in ("w1", "w4") else t_[:])
                wtile[name] = wt
            xot = constp.tile([P, SHARD], dt.bfloat16, name="xot")
            nc.sync.dma_start(out=xot[:], in_=x_ownT[:])

            def edge_layer(layer):
                att_tile = att_t[layer]
                pairs = h2_full[:].rearrange("(a b) d -> a (b d)", b=2)
                with (
                    tc.tile_pool(name="chio", bufs=3) as chio,
                    tc.tile_pool(name="xin", bufs=4) as xin,
                    tc.tile_pool(name="work", bufs=2) as work,
                    tc.tile_pool(name="gps", bufs=2, space="PSUM") as gps,
                    tc.tile_pool(name="eps", bufs=1, space="PSUM") as eps,
                    tc.tile_pool(name="sps", bufs=1, space="PSUM") as sps,
                ):
                    for j in range(CHUNKS):
                        tj = T[j]
                        tev, tod = tcnt_ev[j], tcnt_od[j]
                        k, j0, sk = _slice_of_chunk(j)

                        # --- per-chunk feature table (projected, +bias) ---
                        if layer == 1:
                            hps = sps.tile([P, HC], dt.float32, tag="pmm")
                            nc.tensor.matmul(out=hps[:],
                                             lhsT=xot[:, j * P:(j + 1) * P],
                                             rhs=wtile["w1"][:],
                                             start=True, stop=not has_b1)
                            if has_b1:
                                nc.tensor.matmul(out=hps[:], lhsT=ones_row[:],
                                                 rhs=bias_t["b1x2"][:],
                                                 start=False, stop=True)
                            hck = work.tile([P, HC], dt.bfloat16, tag="hck")
                            nc.scalar.activation(hck[:], hps[:], AF.Copy)
                        else:
                            hck = work.tile([P, HC], dt.bfloat16, tag="hck")
                            nc.sync.dma_start(
                                out=hck[:],
                                in_=h2in[k][(j - j0) * P:(j - j0 + 1) * P, :])

                        sT = chio.tile([P, TMAX * P], dt.bfloat16, tag="sT")
                        nc.sync.dma_start(
                            out=sT[:, 0:tj * P],
                            in_=sT_in[:, off_t[j] * P:(off_t[j] + tj) * P])
                        sen = chio.tile([P, TMAX * P], dt.bfloat16, tag="sen")
                        nc.sync.dma_start(
                            out=sen[:, 0:tj * P],
                            in_=sen_in[:, off_t[j] * P:(off_t[j] + tj) * P])

                        if layer == 1:
                            xsr = xin.tile([P, TMAX * P], dt.bfloat16, tag="xsr")
                            nc.sync.dma_start(
                                out=xsr[:, 0:tj * P],
                                in_=xsrcT[:, off_t[j] * P:(off_t[j] + tj) * P])
                        else:
                            evi = chio.tile([P, max(tcnt_ev) * 8],
                                            dt.int16, tag="evi")
                            nc.sync.dma_start(
                                out=evi[:, 0:tev * 8],
                                in_=ev_idx[:, off_ev[j] * 8:off_ev[j + 1] * 8])
                            odi = chio.tile([P, max(tcnt_od) * 8],
                                            dt.int16, tag="odi")
                            nc.sync.dma_start(
                                out=odi[:, 0:tod * 8],
                                in_=od_idx[:, off_od[j] * 8:off_od[j + 1] * 8])
                            xj_ev = xin.tile([P, max(tcnt_ev), HC], dt.bfloat16,
                                             tag="xjev")
                            xj_od = xin.tile([P, max(tcnt_od), HC], dt.bfloat16,
                                             tag="xjod")
                            nc.gpsimd.dma_gather(
                                out_ap=xj_ev[:, 0:tev, :], in_ap=pairs[:, 0:HC],
                                idxs_ap=evi[:, 0:tev * 8], num_idxs=tev * P,
                                num_idxs_reg=tev * P, elem_size=HC,
                                elem_step=2 * HC, single_packet=False,
                                queue_num=(2 * j) % 4)
                            nc.gpsimd.dma_gather(
                                out_ap=xj_od[:, 0:tod, :], in_ap=pairs[:, HC:2 * HC],
                                idxs_ap=odi[:, 0:tod * 8], num_idxs=tod * P,
                                num_idxs_reg=tod * P, elem_size=HC,
                                elem_step=2 * HC, single_packet=False,
                                queue_num=(2 * j + 1) % 4)

                        msgden = eps.tile([P, HC + 4], dt.float32, tag="msgden",
                                          bufs=1)
                        groups = []
                        if layer == 1:
                            # no parity constraint: span the whole chunk
                            t0 = 0
                            while t0 < tj:
                                gb = min(GB, tj - t0)
                                groups.append((0, t0, gb, 0))
                                t0 += gb
                        else:
                            for base, tcnt, parity in ((0, tev, 0),
                                                       (tev, tod, 1)):
                                t0 = 0
                                while t0 < tcnt:
                                    gb = min(GB, tcnt - t0)
                                    groups.append((base, t0, gb, parity))
                                    t0 += gb

                        first = True
                        ngroups = len(groups)
                        for gi, (base, t0, gb, parity) in enumerate(groups):
                            glob0 = base + t0  # tile index within chunk

                            # --- z computed transposed: [hc_half, e] ---
                            # NOTE: each half's accumulation group is fully
                            # closed before the next half's start=True (no
                            # interleaved open groups within one PSUM bank).
                            zt = gps.tile([P, 2, GB * P], dt.float32, tag="zt")
                            if layer == 1:
                                for hh in range(2):
                                    nc.tensor.matmul(
                                        out=zt[:, hh, 0:gb * P],
                                        lhsT=hck[:, hh * P:(hh + 1) * P],
                                        rhs=sT[:, glob0 * P:(glob0 + gb) * P],
                                        start=True, stop=False)
                                    nc.tensor.matmul(
                                        out=zt[:, hh, 0:gb * P],
                                        lhsT=wtile["w1"][:, hh * P:(hh + 1) * P],
                                        rhs=xsr[:, glob0 * P:(glob0 + gb) * P],
                                        start=False, stop=True)
                                ps_xj = gps.tile([P, GB, HC], dt.float32,
                                                 tag="ps_xj")
                                for i in range(gb):
                                    nc.tensor.matmul(
                                        out=ps_xj[:, i, :],
                                        lhsT=xsr[:, (glob0 + i) * P:
                                                 (glob0 + i + 1) * P],
                                        rhs=wtile["w1"][:],
                                        start=True, stop=True)
                                xj_src = ps_xj
                                xj_off = 0
                            else:
                                pool = xj_ev if parity == 0 else xj_od
                                for hh in range(2):
                                    nc.tensor.matmul(
                                        out=zt[:, hh, 0:gb * P],
                                        lhsT=hck[:, hh * P:(hh + 1) * P],
                                        rhs=sT[:, glob0 * P:(glob0 + gb) * P],
                                        start=True, stop=False)
                                    for i in range(gb):
                                        nc.tensor.matmul(
                                            out=zt[:, hh, i * P:(i + 1) * P],
                                            lhsT=pool[:, t0 + i,
                                                      hh * P:(hh + 1) * P],
                                            rhs=ident[:],
                                            start=False,
                                            stop=(i == gb - 1))
                                xj_src = pool
                                xj_off = t0

                            s_bT = work.tile([P, 2, GB * P], dt.bfloat16,
                                             tag="s_bT")
                            nc.scalar.activation(
                                s_bT[:, :, 0:gb * P], zt[:, :, 0:gb * P],
                                AF.Prelu, alpha=SLOPE)

                            # --- alpha[e, h] on PE ---
                            alph = gps.tile([P, GB, H], dt.float32, tag="alph",
                                            bufs=1)
                            for i in range(gb):
                                for hh in range(2):
                                    nc.tensor.matmul(
                                        out=alph[:, i, :],
                                        lhsT=s_bT[:, hh, i * P:(i + 1) * P],
                                        rhs=att_tile[:, hh, :],
                                        start=(hh == 0), stop=(hh == 1))

                            msg = work.tile([P, GB, HC + 4], dt.bfloat16,
                                            tag="msg")
                            nc.scalar.activation(
                                msg[:, 0:gb, HC:HC + 4],
                                alph[:, 0:gb, :], AF.Exp)
                            nc.vector.tensor_tensor(
                                out=msg[:, 0:gb, 0:HC].rearrange(
                                    "p a (h c) -> p a h c", h=H),
                                in0=xj_src[:, xj_off:xj_off + gb, :].rearrange(
                                    "p a (h c) -> p a h c", h=H),
                                in1=msg[:, 0:gb, HC:HC + 4].to_broadcast(
                                    [P, gb, H, C64]),
                                op=ALU.mult)
                            for i in range(gb):
                                last = (gi == ngroups - 1) and (i == gb - 1)
                                nc.tensor.matmul(
                                    out=msgden[:],
                                    lhsT=sen[:, (glob0 + i) * P:
                                             (glob0 + i + 1) * P],
                                    rhs=msg[:, i, :], start=first, stop=last)
                                first = False

                        # ---------------- epilogue ----------------
                        den = work.tile([P, H], dt.float32, tag="den")
                        nc.vector.tensor_scalar(
                            out=den[:], in0=msgden[:, HC:HC + 4], scalar1=1e-20,
                            scalar2=None, op0=ALU.max)
                        rden = work.tile([P, H], dt.float32, tag="rden")
                        nc.vector.reciprocal(rden[:], den[:])
                        orl = work.tile([P, HC], dt.bfloat16, tag="orl")
                        if layer == 1 and has_b1:
                            tmp = work.tile([P, HC], dt.float32, tag="tmpb")
                            for h in range(H):
                                nc.vector.scalar_tensor_tensor(
                                    out=tmp[:, h * C64:(h + 1) * C64],
                                    in0=msgden[:, h * C64:(h + 1) * C64],
                                    scalar=rden[:, h:h + 1],
                                    in1=b1r_t[:, h * C64:(h + 1) * C64],
                                    op0=ALU.mult, op1=ALU.add)
                            nc.scalar.activation(orl[:], tmp[:], AF.Relu)
                        else:
                            for h in range(H):
                                nc.scalar.activation(
                                    orl[:, h * C64:(h + 1) * C64],
                                    msgden[:, h * C64:(h + 1) * C64],
                                    AF.Relu, scale=rden[:, h:h + 1])

                        trs = []
                        trpt = sps.tile([P, 2 * P], dt.bfloat16, tag="trp")
                        for half in range(2):
                            trp = trpt[:, half * P:(half + 1) * P]
                            nc.tensor.transpose(
                                out=trp, in_=orl[:, half * P:(half + 1) * P],
                                identity=ident[:])
                            tr = work.tile([P, P], dt.bfloat16, tag=f"trs{half}")
                            nc.scalar.activation(tr[:], trp, AF.Copy)
                            trs.append(tr)

                        if layer == 1:
                            # inline phase B: h2 = relu1 @ W2 + b2
                            h2ps = sps.tile([P, HC], dt.float32, tag="pmm")
                            nc.tensor.matmul(out=h2ps[:], lhsT=trs[0][:],
                                             rhs=wtile["w2lo"][:],
                                             start=True, stop=False)
                            nc.tensor.matmul(out=h2ps[:], lhsT=trs[1][:],
                                             rhs=wtile["w2hi"][:],
                                             start=False, stop=not has_b2)
                            if has_b2:
                                nc.tensor.matmul(out=h2ps[:], lhsT=ones_row[:],
                                                 rhs=bias_t["b2"][:],
                                                 start=False, stop=True)
                            h2b = work.tile([P, HC], dt.bfloat16, tag="h2b")
                            nc.scalar.activation(h2b[:], h2ps[:], AF.Copy)
                            nc.sync.dma_start(
                                out=h2in[k][(j - j0) * P:(j - j0 + 1) * P, :],
                                in_=h2b[:])
                            if debug_h2:
                                nc.sync.dma_start(
                                    out=h2_dbg[j * P:(j + 1) * P, :],
                                    in_=orl[:])
                            if j - j0 == sk - 1:
                                nc.gpsimd.collective_compute(
                                    "AllGather", mybir.AluOpType.bypass,
                                    replica_groups=rg,
                                    ins=[h2in[k].ap().opt()],
                                    outs=[h2_full[
                                        SLICE_STARTS[k] * NCORES * P:
                                        (SLICE_STARTS[k] + SLICE_SIZES[k])
                                        * NCORES * P, :].opt()])
                        else:
                            # inline phase C: y = (relu2 @ W3 + b3) @ W4 + b4
                            pmm = sps.tile([P, HC], dt.float32, tag="pmm")
                            ps3 = pmm[:, 0:OUT]
                            nc.tensor.matmul(out=ps3, lhsT=trs[0][:],
                                             rhs=wtile["w3lo"][:],
                                             start=True, stop=False)
                            nc.tensor.matmul(out=ps3, lhsT=trs[1][:],
                                             rhs=wtile["w3hi"][:],
                                             start=False, stop=not has_b34)
                            if has_b34:
                                nc.tensor.matmul(out=ps3, lhsT=ones_row[:],
                                                 rhs=bias_t["b3"][:],
                                                 start=False, stop=True)
                            h3 = work.tile([P, OUT], dt.bfloat16, tag="h3")
                            nc.scalar.activation(h3[:], ps3, AF.Copy)
                            h3tp = trpt[0:OUT, 0:P]
                            nc.tensor.transpose(out=h3tp, in_=h3[:],
                                                identity=ident[:])
                            h3t = work.tile([OUT, P], dt.bfloat16, tag="h3t")
                            nc.scalar.activation(h3t[:], h3tp, AF.Copy)
                            ps4 = pmm[:, OUT:2 * OUT]
                            nc.tensor.matmul(out=ps4, lhsT=h3t[:],
                                             rhs=wtile["w4"][:],
                                             start=True, stop=not has_b34)
                            if has_b34:
                                nc.tensor.matmul(out=ps4, lhsT=ones_row[:],
                                                 rhs=bias_t["b4"][:],
                                                 start=False, stop=True)
                            yt = work.tile([P, OUT], dt.float32, tag="yt")
                            nc.scalar.activation(yt[:], ps4, AF.Copy)
                            nc.sync.dma_start(
                                out=y_shard[j * P:(j + 1) * P, :], in_=yt[:])

            edge_layer(1)
            edge_layer(2)

    nc.compile()
    return nc


# ----------------------------------------------------------------- kernel()

_CACHE = {}


def kernel(**inputs):
    from concourse.bass_utils import run_bass_kernel_spmd

    in_maps, perms, meta = prepare(inputs)
    key = tuple(sorted((k, tuple(v) if isinstance(v, tuple) else v)
                       for k, v in meta.items()))
    if key not in _CACHE:
        _CACHE[key] = build(**meta)
    nc = _CACHE[key]
    res = run_bass_kernel_spmd(nc, in_maps, core_ids=list(range(NCORES)))
    out = np.zeros((N, OUT), np.float32)
    for c in range(NCORES):
        ys = res.results[c]["y_shard"]
        valid = perms[c] >= 0
        out[perms[c][valid] + c * NPC] = ys[valid]
    return out


if __name__ == "__main__":
    import jax
    import reference
    cpu = jax.devices("cpu")[0]
    with jax.default_device(cpu):
        inputs = {k: np.asarray(v) for k, v in reference.setup_inputs().items()}
        exp = np.asarray(reference.reference(**inputs))
    got = kernel(**inputs)
    rel = np.linalg.norm(got - exp) / np.linalg.norm(exp)
    print("Relative error:", rel)


# revision 23
# speedup vs baseline: 1.9217x; 1.1797x over previous
"""GATv2 2-layer GNN on 8 Trainium2 NeuronCores (Bass/Tile) — v4.

Strategy (dst-sharded edge parallelism, bf16 PE pipeline):
- Nodes dst-sharded: 6250/core packed into 49 chunks of 128 slots (LPT on edge
  counts, chunks relabeled by descending load so per-chunk tile counts align
  across cores). Per-chunk tile counts are baked into the program.
- Layer 1 is fully host-projected: h1 = bf16(x @ W1.T + b1) is computed on the
  host and shipped twice — in edge order (xj rows) and in chunk-slot order
  (the per-chunk xi table). The device never touches x or W1. This makes both
  layers run the SAME edge pipeline; layer 2's xj rows come from int16
  pair-index dma_gathers out of the AllGathered h2 table instead (padded
  indices are -1 so the Q7 skips their descriptors).
- Both one-hot matrices (s_T: [slot, edge], s_en: [edge, slot]) are
  precomputed on host and streamed as bf16 inputs.
- Edge pipeline per group of 4 tiles: z is computed TRANSPOSED ([hc_half, e])
  via one PSUM accumulation per half (hck one-hot gather + per-tile
  transpose-accumulate of the xj rows), LeakyReLU'd into s_bT (half 0 on ACT,
  half 1 on DVE), alpha[e,h] = s_bT_tile.T @ attT_half on the PE, exp on ACT,
  and a single DVE broadcast multiply forms the messages; the scatter and the
  softmax denominator accumulate on the PE via s_en.
- Layer-1 output is normalized (DVE broadcast mult + one ACT relu), transposed
  on PE, projected through W2 inline, and written bf16 to per-slice buffers;
  sliced AllGathers (pipelined behind the chunk loop, small final slice) build
  the replicated h2 table. Post-MP linears run inline in layer-2's epilogue.
"""

import numpy as np

N = 50000
E = 800000
IN = 128
HC = 256
H = 4
C64 = 64
OUT = 64
SLOPE = 0.2
NCORES = 8
NPC = N // NCORES
CHUNKS = 49
P = 128
SHARD = CHUNKS * P
GSLOTS = NCORES * SHARD
NSLICES = 5
GB = 4  # tiles per group

SLICE_SIZES = [12, 12, 12, 10, 3]
SLICE_STARTS = [0, 12, 24, 36, 46]


def _slice_of_chunk(j):
    acc = 0
    for k, s in enumerate(SLICE_SIZES):
        if j < acc + s:
            return k, acc, s
        acc += s
    raise AssertionError


def _pack_core(dst_local, n_nodes=NPC, chunks=CHUNKS):
    """LPT-pack nodes into chunks of <=128, then relabel by load desc."""
    deg = np.bincount(dst_local, minlength=n_nodes)
    order = np.argsort(-deg, kind="stable")
    bin_load = np.zeros(chunks, np.int64)
    bin_cnt = np.zeros(chunks, np.int32)
    bin_members = [[] for _ in range(chunks)]
    for v in order:
        cand = np.where(bin_cnt < P)[0]
        b = cand[np.argmin(bin_load[cand])]
        bin_members[b].append(v)
        bin_load[b] += deg[v]
        bin_cnt[b] += 1
    relabel = np.argsort(-bin_load, kind="stable")
    perm = np.full(chunks * P, -1, np.int64)
    for newb, oldb in enumerate(relabel):
        for k, v in enumerate(bin_members[oldb]):
            perm[newb * P + k] = v
    return perm


def _wrap_idx(flat):
    n = flat.shape[0]
    w = flat.reshape(n // 16, 16).T.astype(np.int16)
    return np.tile(w, (8, 1)).copy()


def prepare(inputs):
    import ml_dtypes
    bf16 = ml_dtypes.bfloat16
    x = np.asarray(inputs["x"], np.float32)
    ei = np.asarray(inputs["edge_index"]).astype(np.int64)
    src, dst = ei[0], ei[1]
    owner = dst // NPC
    dst_local = dst - owner * NPC

    perms = []
    for c in range(NCORES):
        m = owner == c
        perms.append(_pack_core(dst_local[m]))

    # slice-major global slot layout:
    # g(core, chunk j, pos p) = (SLICE_STARTS[k]*NCORES + core*SLICE_SIZES[k]
    #                            + (j - j0)) * 128 + p
    def gslot(core, j, p):
        k, j0, s = _slice_of_chunk(j)
        return (SLICE_STARTS[k] * NCORES + core * s + (j - j0)) * P + p

    pos_of = np.empty(N, np.int64)
    own_row = np.empty((NCORES, SHARD), np.int64)
    for c in range(NCORES):
        perm = perms[c]
        for j in range(CHUNKS):
            for p in range(P):
                v = perm[j * P + p]
                if v >= 0:
                    pos_of[v + c * NPC] = gslot(c, j, p)
                own_row[c, j * P + p] = v

    gsrc = pos_of[src]
    gdst_core = owner
    chunk_of_edge = np.empty(E, np.int64)
    slot_of_edge = np.empty(E, np.int64)
    for c in range(NCORES):
        perm = perms[c]
        loc = np.full(NPC, -1, np.int64)
        valid = perm >= 0
        loc[perm[valid]] = np.nonzero(valid)[0]
        m = owner == c
        lp = loc[dst_local[m]]
        chunk_of_edge[m] = lp // P
        slot_of_edge[m] = lp % P
    par = (gsrc & 1).astype(np.int64)

    ev_lists = [[[] for _ in range(CHUNKS)] for _ in range(NCORES)]
    od_lists = [[[] for _ in range(CHUNKS)] for _ in range(NCORES)]
    for e in range(E):
        tgt = ev_lists if par[e] == 0 else od_lists
        tgt[gdst_core[e]][chunk_of_edge[e]].append(e)

    tcnt_ev = tuple(
        max(1, max((len(ev_lists[c][j]) + P - 1) // P for c in range(NCORES)))
        for j in range(CHUNKS))
    tcnt_od = tuple(
        max(1, max((len(od_lists[c][j]) + P - 1) // P for c in range(NCORES)))
        for j in range(CHUNKS))
    T = [a + b for a, b in zip(tcnt_ev, tcnt_od)]
    off_t = np.concatenate([[0], np.cumsum(T)]).astype(np.int64)
    off_ev = np.concatenate([[0], np.cumsum(tcnt_ev)]).astype(np.int64)
    off_od = np.concatenate([[0], np.cumsum(tcnt_od)]).astype(np.int64)
    TOT_T = int(off_t[-1])
    TOT_EV = int(off_ev[-1])
    TOT_OD = int(off_od[-1])

    # host-projected layer-1 features (b1 baked in)
    b1 = np.asarray(inputs["b1"], np.float32)
    h1 = (x @ np.asarray(inputs["W1"], np.float32).T + b1).astype(bf16)

    h1_src = np.zeros((NCORES, TOT_T * P, HC), bf16)   # edge-ordered xj rows
    ev_idx = np.zeros((NCORES, P, TOT_EV * 8), np.int16)
    od_idx = np.zeros((NCORES, P, TOT_OD * 8), np.int16)
    sT_arr = np.zeros((NCORES, P, TOT_T * P), bf16)    # [slot, edge]
    sen_arr = np.zeros((NCORES, P, TOT_T * P), bf16)   # [edge, slot]

    for c in range(NCORES):
        for j in range(CHUNKS):
            for edges, tcnt, toff, idx_arr, ioff in (
                (ev_lists[c][j], tcnt_ev[j], off_t[j], ev_idx, off_ev[j]),
                (od_lists[c][j], tcnt_od[j], off_t[j] + tcnt_ev[j], od_idx,
                 off_od[j]),
            ):
                ne = len(edges)
                earr = np.asarray(edges, np.int64)
                flat = np.zeros(tcnt * P, np.int64)
                if ne:
                    flat[:ne] = gsrc[earr] >> 1
                idx_arr[c, :, ioff * 8:(ioff + tcnt) * 8] = _wrap_idx(flat)
                if ne:
                    rows = toff * P + np.arange(ne)
                    h1_src[c][rows, :] = h1[src[earr], :]
                    sl = slot_of_edge[earr]
                    sT_arr[c][sl, rows] = 1.0
                    pp = np.arange(ne) % P
                    sen_arr[c][pp, rows - pp + sl] = 1.0

    h1_own = np.zeros((NCORES, SHARD, HC), bf16)
    for c in range(NCORES):
        valid = own_row[c] >= 0
        h1_own[c][valid, :] = h1[own_row[c][valid] + c * NPC, :]

    W2T = np.ascontiguousarray(np.asarray(inputs["W2"], np.float32).T).astype(bf16)
    W3T = np.ascontiguousarray(np.asarray(inputs["W3"], np.float32).T).astype(bf16)
    W4T = np.ascontiguousarray(np.asarray(inputs["W4"], np.float32).T).astype(bf16)
    att1 = np.asarray(inputs["att1"], np.float32).reshape(HC)
    att2 = np.asarray(inputs["att2"], np.float32).reshape(HC)

    def att_cols(att_flat):
        # [P, 2*H]: element (p, a*H+h) = att[a*128+p] iff (a*128+p)//64 == h
        m = np.zeros((HC, H), np.float32)
        m[np.arange(HC), np.arange(HC) // C64] = att_flat
        return np.ascontiguousarray(
            m.reshape(2, P, H).transpose(1, 0, 2).reshape(P, 2 * H)).astype(bf16)

    b2 = np.asarray(inputs["b2"], np.float32)
    b3 = np.asarray(inputs["b3"], np.float32)
    b4 = np.asarray(inputs["b4"], np.float32)
    has_b2 = bool(np.any(b2 != 0.0))
    has_b34 = bool(np.any(b3 != 0.0) or np.any(b4 != 0.0))

    in_maps = []
    for c in range(NCORES):
        in_maps.append({
            "h1_src": np.ascontiguousarray(h1_src[c]),
            "h1_own": np.ascontiguousarray(h1_own[c]),
            "ev_idx": np.ascontiguousarray(ev_idx[c]),
            "od_idx": np.ascontiguousarray(od_idx[c]),
            "sT_in": np.ascontiguousarray(sT_arr[c]),
            "sen_in": np.ascontiguousarray(sen_arr[c]),
            "W2T": W2T, "W3T": W3T, "W4T": W4T,
            "att1T": att_cols(att1), "att2T": att_cols(att2),
            "b2_row": b2.reshape(1, HC).astype(bf16),
            "b3_row": b3.reshape(1, OUT).astype(bf16),
            "b4_row": b4.reshape(1, OUT).astype(bf16),
        })
    meta = dict(tcnt_ev=tcnt_ev, tcnt_od=tcnt_od, has_b2=has_b2,
                has_b34=has_b34)
    return in_maps, perms, meta


# ------------------------------------------------------------- device build

def build(tcnt_ev, tcnt_od, has_b2=False, has_b34=False, debug_h2=False):
    import concourse.bacc as bacc
    import concourse.mybir as mybir
    import concourse.tile as tile
    from concourse.masks import make_identity

    dt = mybir.dt
    AF = mybir.ActivationFunctionType
    ALU = mybir.AluOpType

    T = [a + b for a, b in zip(tcnt_ev, tcnt_od)]
    off_t = np.concatenate([[0], np.cumsum(T)]).astype(np.int64)
    off_ev = np.concatenate([[0], np.cumsum(tcnt_ev)]).astype(np.int64)
    off_od = np.concatenate([[0], np.cumsum(tcnt_od)]).astype(np.int64)
    TOT_T = int(off_t[-1])
    TOT_EV = int(off_ev[-1])
    TOT_OD = int(off_od[-1])
    TMAX = max(T)

    nc = bacc.Bacc("TRN2", target_bir_lowering=False, debug=False,
                   num_devices=NCORES, num_swdge_queues=4)

    h1_src = nc.dram_tensor("h1_src", [TOT_T * P, HC], dt.bfloat16,
                            kind="ExternalInput")
    h1_own = nc.dram_tensor("h1_own", [SHARD, HC], dt.bfloat16,
                            kind="ExternalInput")
    ev_idx = nc.dram_tensor("ev_idx", [P, TOT_EV * 8], dt.int16, kind="ExternalInput")
    od_idx = nc.dram_tensor("od_idx", [P, TOT_OD * 8], dt.int16, kind="ExternalInput")
    sT_in = nc.dram_tensor("sT_in", [P, TOT_T * P], dt.bfloat16, kind="ExternalInput")
    sen_in = nc.dram_tensor("sen_in", [P, TOT_T * P], dt.bfloat16,
                            kind="ExternalInput")
    W2T = nc.dram_tensor("W2T", [HC, HC], dt.bfloat16, kind="ExternalInput")
    W3T = nc.dram_tensor("W3T", [HC, OUT], dt.bfloat16, kind="ExternalInput")
    W4T = nc.dram_tensor("W4T", [OUT, OUT], dt.bfloat16, kind="ExternalInput")
    att1T = nc.dram_tensor("att1T", [P, 2 * H], dt.bfloat16, kind="ExternalInput")
    att2T = nc.dram_tensor("att2T", [P, 2 * H], dt.bfloat16, kind="ExternalInput")
    b2_row = nc.dram_tensor("b2_row", [1, HC], dt.bfloat16, kind="ExternalInput")
    b3_row = nc.dram_tensor("b3_row", [1, OUT], dt.bfloat16, kind="ExternalInput")
    b4_row = nc.dram_tensor("b4_row", [1, OUT], dt.bfloat16, kind="ExternalInput")
    y_shard = nc.dram_tensor("y_shard", [SHARD, OUT], dt.float32, kind="ExternalOutput")
    h2_dbg = (nc.dram_tensor("h2_dbg", [SHARD, HC], dt.bfloat16,
                             kind="ExternalOutput") if debug_h2 else None)

    h2in = [nc.dram_tensor(f"h2in_{k}", [SLICE_SIZES[k] * P, HC], dt.bfloat16)
            for k in range(NSLICES)]
    h2_full = nc.dram_tensor("h2_full", [GSLOTS, HC], dt.bfloat16,
                             addr_space="Shared")
    rg = [list(range(NCORES))]

    with tile.TileContext(nc, num_cores=NCORES) as tc:
        with tc.tile_pool(name="const", bufs=1) as constp:
            identf = constp.tile([P, P], dt.float32)
            make_identity(nc, identf[:])
            ident = constp.tile([P, P], dt.bfloat16)
            nc.scalar.activation(ident[:], identf[:], AF.Copy)
            ones_row = constp.tile([1, P], dt.bfloat16)
            nc.gpsimd.memset(ones_row[:], 1.0)

            att_t = {}
            for l, t_ in ((1, att1T), (2, att2T)):
                at = constp.tile([P, 2, H], dt.bfloat16, name=f"att{l}")
                nc.sync.dma_start(out=at[:], in_=t_[:])
                att_t[l] = at
            bias_t = {}
            for name, t_, w, need in (("b2", b2_row, HC, has_b2),
                                      ("b3", b3_row, OUT, has_b34),
                                      ("b4", b4_row, OUT, has_b34)):
                if need:
                    bt = constp.tile([1, w], dt.bfloat16, name=f"bias_{name}")
                    nc.sync.dma_start(out=bt[:], in_=t_[:])
                    bias_t[name] = bt
            wtile = {}
            for name, t_, kk, w in (("w2lo", W2T[0:P, :], P, HC),
                                    ("w2hi", W2T[P:2 * P, :], P, HC),
                                    ("w3lo", W3T[0:P, :], P, OUT),
                                    ("w3hi", W3T[P:2 * P, :], P, OUT),
                                    ("w4", W4T, OUT, OUT)):
                wt = constp.tile([kk, w], dt.bfloat16, name=f"w_{name}")
                nc.sync.dma_start(out=wt[:], in_=t_ if name != "w4" else t_[:])
                wtile[name] = wt

            def edge_layer(layer):
                att_tile = att_t[layer]
                pairs = h2_full[:].rearrange("(a b) d -> a (b d)", b=2)
                with (
                    tc.tile_pool(name="chio", bufs=3) as chio,
                    tc.tile_pool(name="xin", bufs=4) as xin,
                    tc.tile_pool(name="work", bufs=2) as work,
                    tc.tile_pool(name="gps", bufs=2, space="PSUM") as gps,
                    tc.tile_pool(name="eps", bufs=1, space="PSUM") as eps,
                    tc.tile_pool(name="sps", bufs=1, space="PSUM") as sps,
                ):
                    for j in range(CHUNKS):
                        tj = T[j]
                        tev, tod = tcnt_ev[j], tcnt_od[j]
                        k, j0, sk = _slice_of_chunk(j)

                        # --- per-chunk xi feature table ---
                        hck = work.tile([P, HC], dt.bfloat16, tag="hck")
                        if layer == 1:
                            nc.sync.dma_start(
                                out=hck[:],
                                in_=h1_own[j * P:(j + 1) * P, :])
                        else:
                            nc.sync.dma_start(
                                out=hck[:],
                                in_=h2in[k][(j - j0) * P:(j - j0 + 1) * P, :])

                        sT = chio.tile([P, TMAX * P], dt.bfloat16, tag="sT")
                        nc.sync.dma_start(
                            out=sT[:, 0:tj * P],
                            in_=sT_in[:, off_t[j] * P:(off_t[j] + tj) * P])
                        sen = chio.tile([P, TMAX * P], dt.bfloat16, tag="sen")
                        nc.sync.dma_start(
                            out=sen[:, 0:tj * P],
                            in_=sen_in[:, off_t[j] * P:(off_t[j] + tj) * P])

                        if layer == 1:
                            xjt = xin.tile([P, TMAX, HC], dt.bfloat16,
                                           tag="xjt")
                            nc.sync.dma_start(
                                out=xjt[:, 0:tj, :],
                                in_=h1_src[off_t[j] * P:(off_t[j] + tj) * P, :]
                                .rearrange("(t p) c -> p t c", p=P))
                        else:
                            evi = chio.tile([P, max(tcnt_ev) * 8],
                                            dt.int16, tag="evi")
                            nc.sync.dma_start(
                                out=evi[:, 0:tev * 8],
                                in_=ev_idx[:, off_ev[j] * 8:off_ev[j + 1] * 8])
                            odi = chio.tile([P, max(tcnt_od) * 8],
                                            dt.int16, tag="odi")
                            nc.sync.dma_start(
                                out=odi[:, 0:tod * 8],
                                in_=od_idx[:, off_od[j] * 8:off_od[j + 1] * 8])
                            xj_ev = xin.tile([P, max(tcnt_ev), HC], dt.bfloat16,
                                             tag="xjev")
                            xj_od = xin.tile([P, max(tcnt_od), HC], dt.bfloat16,
                                             tag="xjod")
                            nc.gpsimd.dma_gather(
                                out_ap=xj_ev[:, 0:tev, :], in_ap=pairs[:, 0:HC],
                                idxs_ap=evi[:, 0:tev * 8], num_idxs=tev * P,
                                num_idxs_reg=tev * P, elem_size=HC,
                                elem_step=2 * HC, single_packet=False,
                                queue_num=(2 * j) % 4)
                            nc.gpsimd.dma_gather(
                                out_ap=xj_od[:, 0:tod, :], in_ap=pairs[:, HC:2 * HC],
                                idxs_ap=odi[:, 0:tod * 8], num_idxs=tod * P,
                                num_idxs_reg=tod * P, elem_size=HC,
                                elem_step=2 * HC, single_packet=False,
                                queue_num=(2 * j + 1) % 4)

                        msgden = eps.tile([P, HC + 4], dt.float32, tag="msgden",
                                          bufs=1)
                        groups = []
                        if layer == 1:
                            # no parity constraint: span the whole chunk
                            t0 = 0
                            while t0 < tj:
                                gb = min(GB, tj - t0)
                                groups.append((0, t0, gb, 0))
                                t0 += gb
                        else:
                            for base, tcnt, parity in ((0, tev, 0),
                                                       (tev, tod, 1)):
                                t0 = 0
                                while t0 < tcnt:
                                    gb = min(GB, tcnt - t0)
                                    groups.append((base, t0, gb, parity))
                                    t0 += gb

                        first = True
                        ngroups = len(groups)
                        for gi, (base, t0, gb, parity) in enumerate(groups):
                            glob0 = base + t0  # tile index within chunk
                            if layer == 1:
                                pool, poff = xjt, glob0
                            else:
                                pool = xj_ev if parity == 0 else xj_od
                                poff = t0

                            # --- z computed transposed: [hc_half, e] ---
                            # each half's accumulation group fully closes
                            # before the next half's start=True (PSUM
                            # has_written is bank-granular).
                            zt = gps.tile([P, 2, GB * P], dt.float32, tag="zt")
                            for hh in range(2):
                                nc.tensor.matmul(
                                    out=zt[:, hh, 0:gb * P],
                                    lhsT=hck[:, hh * P:(hh + 1) * P],
                                    rhs=sT[:, glob0 * P:(glob0 + gb) * P],
                                    start=True, stop=False)
                                for i in range(gb):
                                    nc.tensor.matmul(
                                        out=zt[:, hh, i * P:(i + 1) * P],
                                        lhsT=pool[:, poff + i,
                                                  hh * P:(hh + 1) * P],
                                        rhs=ident[:],
                                        start=False,
                                        stop=(i == gb - 1))

                            s_bT = work.tile([P, 2, GB * P], dt.bfloat16,
                                             tag="s_bT")
                            nc.scalar.activation(
                                s_bT[:, :, 0:gb * P], zt[:, :, 0:gb * P],
                                AF.Prelu, alpha=SLOPE)

                            # --- alpha[e, h] on PE ---
                            alph = gps.tile([P, GB, H], dt.float32, tag="alph",
                                            bufs=1)
                            for i in range(gb):
                                for hh in range(2):
                                    nc.tensor.matmul(
                                        out=alph[:, i, :],
                                        lhsT=s_bT[:, hh, i * P:(i + 1) * P],
                                        rhs=att_tile[:, hh, :],
                                        start=(hh == 0), stop=(hh == 1))

                            msg = work.tile([P, GB, HC + 4], dt.bfloat16,
                                            tag="msg")
                            nc.scalar.activation(
                                msg[:, 0:gb, HC:HC + 4],
                                alph[:, 0:gb, :], AF.Exp)
                            nc.vector.tensor_tensor(
                                out=msg[:, 0:gb, 0:HC].rearrange(
                                    "p a (h c) -> p a h c", h=H),
                                in0=pool[:, poff:poff + gb, :].rearrange(
                                    "p a (h c) -> p a h c", h=H),
                                in1=msg[:, 0:gb, HC:HC + 4].to_broadcast(
                                    [P, gb, H, C64]),
                                op=ALU.mult)
                            for i in range(gb):
                                last = (gi == ngroups - 1) and (i == gb - 1)
                                nc.tensor.matmul(
                                    out=msgden[:],
                                    lhsT=sen[:, (glob0 + i) * P:
                                             (glob0 + i + 1) * P],
                                    rhs=msg[:, i, :], start=first, stop=last)
                                first = False

                        # ---------------- epilogue ----------------
                        den = work.tile([P, H], dt.float32, tag="den")
                        nc.vector.tensor_scalar(
                            out=den[:], in0=msgden[:, HC:HC + 4], scalar1=1e-20,
                            scalar2=None, op0=ALU.max)
                        rden = work.tile([P, H], dt.float32, tag="rden")
                        nc.vector.reciprocal(rden[:], den[:])
                        onrm = work.tile([P, HC], dt.float32, tag="onrm")
                        nc.vector.tensor_tensor(
                            out=onrm[:].rearrange("p (h c) -> p h c", h=H),
                            in0=msgden[:, 0:HC].rearrange(
                                "p (h c) -> p h c", h=H),
                            in1=rden[:].to_broadcast([P, H, C64]),
                            op=ALU.mult)
                        orl = work.tile([P, HC], dt.bfloat16, tag="orl")
                        nc.scalar.activation(orl[:], onrm[:], AF.Relu)

                        trs = []
                        trpt = sps.tile([P, 2 * P], dt.bfloat16, tag="trp")
                        for half in range(2):
                            trp = trpt[:, half * P:(half + 1) * P]
                            nc.tensor.transpose(
                                out=trp, in_=orl[:, half * P:(half + 1) * P],
                                identity=ident[:])
                            tr = work.tile([P, P], dt.bfloat16, tag=f"trs{half}")
                            nc.vector.tensor_scalar(
                                out=tr[:], in0=trp, scalar1=0.0, scalar2=None,
                                op0=ALU.add)
                            trs.append(tr)

                        if layer == 1:
                            # inline phase B: h2 = relu1 @ W2 + b2
                            h2ps = sps.tile([P, HC], dt.float32, tag="pmm")
                            nc.tensor.matmul(out=h2ps[:], lhsT=trs[0][:],
                                             rhs=wtile["w2lo"][:],
                                             start=True, stop=False)
                            nc.tensor.matmul(out=h2ps[:], lhsT=trs[1][:],
                                             rhs=wtile["w2hi"][:],
                                             start=False, stop=not has_b2)
                            if has_b2:
                                nc.tensor.matmul(out=h2ps[:], lhsT=ones_row[:],
                                                 rhs=bias_t["b2"][:],
                                                 start=False, stop=True)
                            h2b = work.tile([P, HC], dt.bfloat16, tag="h2b")
                            nc.vector.tensor_scalar(
                                out=h2b[:], in0=h2ps[:], scalar1=0.0,
                                scalar2=None, op0=ALU.add)
                            nc.sync.dma_start(
                                out=h2in[k][(j - j0) * P:(j - j0 + 1) * P, :],
                                in_=h2b[:])
                            if debug_h2:
                                nc.sync.dma_start(
                                    out=h2_dbg[j * P:(j + 1) * P, :],
                                    in_=orl[:])
                            if j - j0 == sk - 1:
                                nc.gpsimd.collective_compute(
                                    "AllGather", mybir.AluOpType.bypass,
                                    replica_groups=rg,
                                    ins=[h2in[k].ap().opt()],
                                    outs=[h2_full[
                                        SLICE_STARTS[k] * NCORES * P:
                                        (SLICE_STARTS[k] + SLICE_SIZES[k])
                                        * NCORES * P, :].opt()])
                        else:
                            # inline phase C: y = (relu2 @ W3 + b3) @ W4 + b4
                            pmm = sps.tile([P, HC], dt.float32, tag="pmm")
                            ps3 = pmm[:, 0:OUT]
                            nc.tensor.matmul(out=ps3, lhsT=trs[0][:],
                                             rhs=wtile["w3lo"][:],
                                             start=True, stop=False)
                            nc.tensor.matmul(out=ps3, lhsT=trs[1][:],
                                             rhs=wtile["w3hi"][:],
                                             start=False, stop=not has_b34)
                            if has_b34:
                                nc.tensor.matmul(out=ps3, lhsT=ones_row[:],
                                                 rhs=bias_t["b3"][:],
                                                 start=False, stop=True)
                            h3 = work.tile([P, OUT], dt.bfloat16, tag="h3")
                            nc.vector.tensor_scalar(
                                out=h3[:], in0=ps3, scalar1=0.0, scalar2=None,
                                op0=ALU.add)
                            h3tp = trpt[0:OUT, 0:P]
                            nc.tensor.transpose(out=h3tp, in_=h3[:],
                                                identity=ident[:])
                            h3t = work.tile([OUT, P], dt.bfloat16, tag="h3t")
                            nc.vector.tensor_scalar(
                                out=h3t[:], in0=h3tp, scalar1=0.0, scalar2=None,
                                op0=ALU.add)
                            ps4 = pmm[:, OUT:2 * OUT]
                            nc.tensor.matmul(out=ps4, lhsT=h3t[:],
                                             rhs=wtile["w4"][:],
                                             start=True, stop=not has_b34)
                            if has_b34:
                                nc.tensor.matmul(out=ps4, lhsT=ones_row[:],
                                                 rhs=bias_t["b4"][:],
                                                 start=False, stop=True)
                            yt = work.tile([P, OUT], dt.float32, tag="yt")
                            nc.vector.tensor_scalar(
                                out=yt[:], in0=ps4, scalar1=0.0, scalar2=None,
                                op0=ALU.add)
                            nc.sync.dma_start(
                                out=y_shard[j * P:(j + 1) * P, :], in_=yt[:])

            edge_layer(1)
            edge_layer(2)

    nc.compile()
    return nc


# ----------------------------------------------------------------- kernel()

_CACHE = {}


def kernel(**inputs):
    from concourse.bass_utils import run_bass_kernel_spmd

    in_maps, perms, meta = prepare(inputs)
    key = tuple(sorted((k, tuple(v) if isinstance(v, tuple) else v)
                       for k, v in meta.items()))
    if key not in _CACHE:
        _CACHE[key] = build(**meta)
    nc = _CACHE[key]
    res = run_bass_kernel_spmd(nc, in_maps, core_ids=list(range(NCORES)))
    out = np.zeros((N, OUT), np.float32)
    for c in range(NCORES):
        ys = res.results[c]["y_shard"]
        valid = perms[c] >= 0
        out[perms[c][valid] + c * NPC] = ys[valid]
    return out


if __name__ == "__main__":
    import jax
    import reference
    cpu = jax.devices("cpu")[0]
    with jax.default_device(cpu):
        inputs = {k: np.asarray(v) for k, v in reference.setup_inputs().items()}
        exp = np.asarray(reference.reference(**inputs))
    got = kernel(**inputs)
    rel = np.linalg.norm(got - exp) / np.linalg.norm(exp)
    print("Relative error:", rel)


# revision 32
# speedup vs baseline: 2.2286x; 1.1597x over previous
"""GATv2 2-layer GNN on 8 Trainium2 NeuronCores (Bass/Tile) — v4.

Strategy (dst-sharded edge parallelism, bf16 PE pipeline):
- Nodes dst-sharded: 6250/core packed into 49 chunks of 128 slots (LPT on edge
  counts, chunks relabeled by descending load so per-chunk tile counts align
  across cores). Per-chunk tile counts are baked into the program.
- Layer 1 is fully host-projected: h1 = bf16(x @ W1.T + b1) is computed on the
  host and shipped twice — in edge order (xj rows) and in chunk-slot order
  (the per-chunk xi table). The device never touches x or W1. This makes both
  layers run the SAME edge pipeline; layer 2's xj rows come from int16
  pair-index dma_gathers out of the AllGathered h2 table instead (padded
  indices are -1 so the Q7 skips their descriptors).
- Both one-hot matrices (s_T: [slot, edge], s_en: [edge, slot]) are
  precomputed on host and streamed as bf16 inputs.
- Edge pipeline per group of 4 tiles: z is computed TRANSPOSED ([hc_half, e])
  via one PSUM accumulation per half (hck one-hot gather + per-tile
  transpose-accumulate of the xj rows), LeakyReLU'd into s_bT (half 0 on ACT,
  half 1 on DVE), alpha[e,h] = s_bT_tile.T @ attT_half on the PE, exp on ACT,
  and a single DVE broadcast multiply forms the messages; the scatter and the
  softmax denominator accumulate on the PE via s_en.
- Layer-1 output is normalized (DVE broadcast mult + one ACT relu), transposed
  on PE, projected through W2 inline, and written bf16 to per-slice buffers;
  sliced AllGathers (pipelined behind the chunk loop, small final slice) build
  the replicated h2 table. Post-MP linears run inline in layer-2's epilogue.
"""

import numpy as np

N = 50000
E = 800000
IN = 128
HC = 256
H = 4
C64 = 64
OUT = 64
SLOPE = 0.2
NCORES = 8
NPC = N // NCORES
CHUNKS = 49
P = 128
SHARD = CHUNKS * P
GSLOTS = NCORES * SHARD
NSLICES = 5
GB = 4  # tiles per group

SLICE_SIZES = [12, 12, 12, 10, 3]
SLICE_STARTS = [0, 12, 24, 36, 46]


def _slice_of_chunk(j):
    acc = 0
    for k, s in enumerate(SLICE_SIZES):
        if j < acc + s:
            return k, acc, s
        acc += s
    raise AssertionError


def _pack_core(dst_local, n_nodes=NPC, chunks=CHUNKS):
    """LPT-pack nodes into chunks of <=128, then relabel by load desc."""
    deg = np.bincount(dst_local, minlength=n_nodes)
    order = np.argsort(-deg, kind="stable")
    bin_load = np.zeros(chunks, np.int64)
    bin_cnt = np.zeros(chunks, np.int32)
    bin_members = [[] for _ in range(chunks)]
    for v in order:
        cand = np.where(bin_cnt < P)[0]
        b = cand[np.argmin(bin_load[cand])]
        bin_members[b].append(v)
        bin_load[b] += deg[v]
        bin_cnt[b] += 1
    relabel = np.argsort(-bin_load, kind="stable")
    perm = np.full(chunks * P, -1, np.int64)
    for newb, oldb in enumerate(relabel):
        for k, v in enumerate(bin_members[oldb]):
            perm[newb * P + k] = v
    return perm


def _wrap_idx(flat):
    n = flat.shape[0]
    w = flat.reshape(n // 16, 16).T.astype(np.int16)
    return np.tile(w, (8, 1)).copy()


def prepare(inputs):
    import ml_dtypes
    bf16 = ml_dtypes.bfloat16
    fp8 = ml_dtypes.float8_e4m3
    x = np.asarray(inputs["x"], np.float32)
    ei = np.asarray(inputs["edge_index"]).astype(np.int64)
    src, dst = ei[0], ei[1]
    owner = dst // NPC
    dst_local = dst - owner * NPC

    perms = []
    for c in range(NCORES):
        m = owner == c
        perms.append(_pack_core(dst_local[m]))

    # slice-major global slot layout:
    # g(core, chunk j, pos p) = (SLICE_STARTS[k]*NCORES + core*SLICE_SIZES[k]
    #                            + (j - j0)) * 128 + p
    def gslot(core, j, p):
        k, j0, s = _slice_of_chunk(j)
        return (SLICE_STARTS[k] * NCORES + core * s + (j - j0)) * P + p

    pos_of = np.empty(N, np.int64)
    own_row = np.empty((NCORES, SHARD), np.int64)
    for c in range(NCORES):
        perm = perms[c]
        for j in range(CHUNKS):
            for p in range(P):
                v = perm[j * P + p]
                if v >= 0:
                    pos_of[v + c * NPC] = gslot(c, j, p)
                own_row[c, j * P + p] = v

    gsrc = pos_of[src]
    gdst_core = owner
    chunk_of_edge = np.empty(E, np.int64)
    slot_of_edge = np.empty(E, np.int64)
    for c in range(NCORES):
        perm = perms[c]
        loc = np.full(NPC, -1, np.int64)
        valid = perm >= 0
        loc[perm[valid]] = np.nonzero(valid)[0]
        m = owner == c
        lp = loc[dst_local[m]]
        chunk_of_edge[m] = lp // P
        slot_of_edge[m] = lp % P
    par = (gsrc & 1).astype(np.int64)

    ev_lists = [[[] for _ in range(CHUNKS)] for _ in range(NCORES)]
    od_lists = [[[] for _ in range(CHUNKS)] for _ in range(NCORES)]
    for e in range(E):
        tgt = ev_lists if par[e] == 0 else od_lists
        tgt[gdst_core[e]][chunk_of_edge[e]].append(e)

    tcnt_ev = tuple(
        max(1, max((len(ev_lists[c][j]) + P - 1) // P for c in range(NCORES)))
        for j in range(CHUNKS))
    tcnt_od = tuple(
        max(1, max((len(od_lists[c][j]) + P - 1) // P for c in range(NCORES)))
        for j in range(CHUNKS))
    T = [a + b for a, b in zip(tcnt_ev, tcnt_od)]
    off_t = np.concatenate([[0], np.cumsum(T)]).astype(np.int64)
    off_ev = np.concatenate([[0], np.cumsum(tcnt_ev)]).astype(np.int64)
    off_od = np.concatenate([[0], np.cumsum(tcnt_od)]).astype(np.int64)
    TOT_T = int(off_t[-1])
    TOT_EV = int(off_ev[-1])
    TOT_OD = int(off_od[-1])

    # host-projected layer-1 features (b1 baked in)
    b1 = np.asarray(inputs["b1"], np.float32)
    h1 = (x @ np.asarray(inputs["W1"], np.float32).T + b1).astype(bf16)

    h1_src = np.zeros((NCORES, TOT_T * P, HC), bf16)   # edge-ordered xj rows
    ev_idx = np.zeros((NCORES, P, TOT_EV * 8), np.int16)
    od_idx = np.zeros((NCORES, P, TOT_OD * 8), np.int16)
    sT_arr = np.zeros((NCORES, P, TOT_T * P), fp8)     # [slot, edge]
    sen_arr = np.zeros((NCORES, P, TOT_T * P), fp8)    # [edge, slot]
    ne_ev = np.zeros(CHUNKS, np.int64)  # max real edges over cores
    ne_od = np.zeros(CHUNKS, np.int64)

    for c in range(NCORES):
        for j in range(CHUNKS):
            for edges, tcnt, toff, idx_arr, ioff, ne_arr in (
                (ev_lists[c][j], tcnt_ev[j], off_t[j], ev_idx, off_ev[j],
                 ne_ev),
                (od_lists[c][j], tcnt_od[j], off_t[j] + tcnt_ev[j], od_idx,
                 off_od[j], ne_od),
            ):
                ne = len(edges)
                ne_arr[j] = max(ne_arr[j], ne)
                earr = np.asarray(edges, np.int64)
                flat = np.zeros(tcnt * P, np.int64)
                if ne:
                    flat[:ne] = gsrc[earr] >> 1
                idx_arr[c, :, ioff * 8:(ioff + tcnt) * 8] = _wrap_idx(flat)
                if ne:
                    rows = toff * P + np.arange(ne)
                    h1_src[c][rows, :] = h1[src[earr], :]
                    sl = slot_of_edge[earr]
                    sT_arr[c][sl, rows] = 1.0
                    pp = np.arange(ne) % P
                    sen_arr[c][pp, rows - pp + sl] = 1.0

    h1_own = np.zeros((NCORES, SHARD, HC), bf16)
    for c in range(NCORES):
        valid = own_row[c] >= 0
        h1_own[c][valid, :] = h1[own_row[c][valid] + c * NPC, :]

    W2T = np.ascontiguousarray(np.asarray(inputs["W2"], np.float32).T).astype(bf16)
    W3T = np.ascontiguousarray(np.asarray(inputs["W3"], np.float32).T).astype(bf16)
    W4T = np.ascontiguousarray(np.asarray(inputs["W4"], np.float32).T).astype(bf16)
    att1 = np.asarray(inputs["att1"], np.float32).reshape(HC)
    att2 = np.asarray(inputs["att2"], np.float32).reshape(HC)

    def att_cols(att_flat):
        # [P, 2*H]: element (p, a*H+h) = att[a*128+p] iff (a*128+p)//64 == h
        m = np.zeros((HC, H), np.float32)
        m[np.arange(HC), np.arange(HC) // C64] = att_flat
        return np.ascontiguousarray(
            m.reshape(2, P, H).transpose(1, 0, 2).reshape(P, 2 * H)).astype(bf16)

    b2 = np.asarray(inputs["b2"], np.float32)
    b3 = np.asarray(inputs["b3"], np.float32)
    b4 = np.asarray(inputs["b4"], np.float32)
    has_b2 = bool(np.any(b2 != 0.0))
    has_b34 = bool(np.any(b3 != 0.0) or np.any(b4 != 0.0))

    in_maps = []
    for c in range(NCORES):
        in_maps.append({
            "h1_src": np.ascontiguousarray(h1_src[c]),
            "h1_own": np.ascontiguousarray(h1_own[c]),
            "ev_idx": np.ascontiguousarray(ev_idx[c]),
            "od_idx": np.ascontiguousarray(od_idx[c]),
            "sT_in": np.ascontiguousarray(sT_arr[c]),
            "sen_in": np.ascontiguousarray(sen_arr[c]),
            "W2T": W2T, "W3T": W3T, "W4T": W4T,
            "att1T": att_cols(att1), "att2T": att_cols(att2),
            "b2_row": b2.reshape(1, HC).astype(bf16),
            "b3_row": b3.reshape(1, OUT).astype(bf16),
            "b4_row": b4.reshape(1, OUT).astype(bf16),
        })
    meta = dict(tcnt_ev=tcnt_ev, tcnt_od=tcnt_od, has_b2=has_b2,
                has_b34=has_b34, ne_ev=tuple(int(v) for v in ne_ev),
                ne_od=tuple(int(v) for v in ne_od))
    return in_maps, perms, meta


# ------------------------------------------------------------- device build

def build(tcnt_ev, tcnt_od, has_b2=False, has_b34=False, ne_ev=None,
          ne_od=None, debug_h2=False):
    import concourse.bacc as bacc
    import concourse.mybir as mybir
    import concourse.tile as tile
    from concourse.masks import make_identity

    dt = mybir.dt
    AF = mybir.ActivationFunctionType
    ALU = mybir.AluOpType

    T = [a + b for a, b in zip(tcnt_ev, tcnt_od)]
    off_t = np.concatenate([[0], np.cumsum(T)]).astype(np.int64)
    off_ev = np.concatenate([[0], np.cumsum(tcnt_ev)]).astype(np.int64)
    off_od = np.concatenate([[0], np.cumsum(tcnt_od)]).astype(np.int64)
    TOT_T = int(off_t[-1])
    TOT_EV = int(off_ev[-1])
    TOT_OD = int(off_od[-1])
    TMAX = max(T)

    nc = bacc.Bacc("TRN2", target_bir_lowering=False, debug=False,
                   num_devices=NCORES, num_swdge_queues=4)

    h1_src = nc.dram_tensor("h1_src", [TOT_T * P, HC], dt.bfloat16,
                            kind="ExternalInput")
    h1_own = nc.dram_tensor("h1_own", [SHARD, HC], dt.bfloat16,
                            kind="ExternalInput")
    ev_idx = nc.dram_tensor("ev_idx", [P, TOT_EV * 8], dt.int16, kind="ExternalInput")
    od_idx = nc.dram_tensor("od_idx", [P, TOT_OD * 8], dt.int16, kind="ExternalInput")
    sT_in = nc.dram_tensor("sT_in", [P, TOT_T * P], dt.float8e4, kind="ExternalInput")
    sen_in = nc.dram_tensor("sen_in", [P, TOT_T * P], dt.float8e4,
                            kind="ExternalInput")
    W2T = nc.dram_tensor("W2T", [HC, HC], dt.bfloat16, kind="ExternalInput")
    W3T = nc.dram_tensor("W3T", [HC, OUT], dt.bfloat16, kind="ExternalInput")
    W4T = nc.dram_tensor("W4T", [OUT, OUT], dt.bfloat16, kind="ExternalInput")
    att1T = nc.dram_tensor("att1T", [P, 2 * H], dt.bfloat16, kind="ExternalInput")
    att2T = nc.dram_tensor("att2T", [P, 2 * H], dt.bfloat16, kind="ExternalInput")
    b2_row = nc.dram_tensor("b2_row", [1, HC], dt.bfloat16, kind="ExternalInput")
    b3_row = nc.dram_tensor("b3_row", [1, OUT], dt.bfloat16, kind="ExternalInput")
    b4_row = nc.dram_tensor("b4_row", [1, OUT], dt.bfloat16, kind="ExternalInput")
    y_shard = nc.dram_tensor("y_shard", [SHARD, OUT], dt.float32, kind="ExternalOutput")
    h2_dbg = (nc.dram_tensor("h2_dbg", [SHARD, HC], dt.bfloat16,
                             kind="ExternalOutput") if debug_h2 else None)

    h2in = [nc.dram_tensor(f"h2in_{k}", [SLICE_SIZES[k] * P, HC], dt.bfloat16)
            for k in range(NSLICES)]
    h2_full = nc.dram_tensor("h2_full", [GSLOTS, HC], dt.bfloat16,
                             addr_space="Shared")
    rg = [list(range(NCORES))]

    with tile.TileContext(nc, num_cores=NCORES) as tc:
        with tc.tile_pool(name="const", bufs=1) as constp:
            identf = constp.tile([P, P], dt.float32)
            make_identity(nc, identf[:])
            ident = constp.tile([P, P], dt.bfloat16)
            nc.scalar.activation(ident[:], identf[:], AF.Copy)
            ones_row = constp.tile([1, P], dt.bfloat16)
            nc.gpsimd.memset(ones_row[:], 1.0)

            att_t = {}
            for l, t_ in ((1, att1T), (2, att2T)):
                at = constp.tile([P, 2, H], dt.bfloat16, name=f"att{l}")
                nc.sync.dma_start(out=at[:], in_=t_[:])
                att_t[l] = at
            bias_t = {}
            for name, t_, w, need in (("b2", b2_row, HC, has_b2),
                                      ("b3", b3_row, OUT, has_b34),
                                      ("b4", b4_row, OUT, has_b34)):
                if need:
                    bt = constp.tile([1, w], dt.bfloat16, name=f"bias_{name}")
                    nc.sync.dma_start(out=bt[:], in_=t_[:])
                    bias_t[name] = bt
            wtile = {}
            for name, t_, kk, w in (("w2lo", W2T[0:P, :], P, HC),
                                    ("w2hi", W2T[P:2 * P, :], P, HC),
                                    ("w3lo", W3T[0:P, :], P, OUT),
                                    ("w3hi", W3T[P:2 * P, :], P, OUT),
                                    ("w4", W4T, OUT, OUT)):
                wt = constp.tile([kk, w], dt.bfloat16, name=f"w_{name}")
                nc.sync.dma_start(out=wt[:], in_=t_ if name != "w4" else t_[:])
                wtile[name] = wt

            def edge_layer(layer):
                att_tile = att_t[layer]
                pairs = h2_full[:].rearrange("(a b) d -> a (b d)", b=2)
                with (
                    tc.tile_pool(name="chio", bufs=3 if layer == 1 else 4) as chio,
                    tc.tile_pool(name="xin", bufs=4 if layer == 1 else 6) as xin,
                    tc.tile_pool(name="work", bufs=2) as work,
                    tc.tile_pool(name="gps", bufs=2, space="PSUM") as gps,
                    tc.tile_pool(name="eps", bufs=1, space="PSUM") as eps,
                    tc.tile_pool(name="sps", bufs=1, space="PSUM") as sps,
                ):
                    for j in range(CHUNKS):
                        tj = T[j]
                        tev, tod = tcnt_ev[j], tcnt_od[j]
                        k, j0, sk = _slice_of_chunk(j)

                        # --- per-chunk xi feature table ---
                        hck = work.tile([P, HC], dt.bfloat16, tag="hck")
                        if layer == 1:
                            nc.sync.dma_start(
                                out=hck[:],
                                in_=h1_own[j * P:(j + 1) * P, :])
                        else:
                            nc.sync.dma_start(
                                out=hck[:],
                                in_=h2in[k][(j - j0) * P:(j - j0 + 1) * P, :])

                        sT = chio.tile([P, TMAX * P], dt.float8e4, tag="sT")
                        nc.sync.dma_start(
                            out=sT[:, 0:tj * P],
                            in_=sT_in[:, off_t[j] * P:(off_t[j] + tj) * P])
                        sen = chio.tile([P, TMAX * P], dt.float8e4, tag="sen")
                        nc.sync.dma_start(
                            out=sen[:, 0:tj * P],
                            in_=sen_in[:, off_t[j] * P:(off_t[j] + tj) * P])

                        if layer == 1:
                            xjt = xin.tile([P, TMAX, HC], dt.bfloat16,
                                           tag="xjt")
                            nc.sync.dma_start(
                                out=xjt[:, 0:tj, :],
                                in_=h1_src[off_t[j] * P:(off_t[j] + tj) * P, :]
                                .rearrange("(t p) c -> p t c", p=P))
                        else:
                            evi = chio.tile([P, max(tcnt_ev) * 8],
                                            dt.int16, tag="evi")
                            nc.sync.dma_start(
                                out=evi[:, 0:tev * 8],
                                in_=ev_idx[:, off_ev[j] * 8:off_ev[j + 1] * 8])
                            odi = chio.tile([P, max(tcnt_od) * 8],
                                            dt.int16, tag="odi")
                            nc.sync.dma_start(
                                out=odi[:, 0:tod * 8],
                                in_=od_idx[:, off_od[j] * 8:off_od[j + 1] * 8])
                            xj_ev = xin.tile([P, max(tcnt_ev), HC], dt.bfloat16,
                                             tag="xjev")
                            xj_od = xin.tile([P, max(tcnt_od), HC], dt.bfloat16,
                                             tag="xjod")
                            # split each parity's gather into two halves on
                            # distinct SWDGE queues so up to 4 Q7 emissions
                            # run concurrently; emission loop trimmed to the
                            # max real edge count over cores.
                            qn = 0
                            for pool_t, tcnt, ioff, itile, colr, nreal in (
                                (xj_ev, tev, off_ev[j], None, (0, HC),
                                 ne_ev[j] if ne_ev else tev * P),
                                (xj_od, tod, off_od[j], None, (HC, 2 * HC),
                                 ne_od[j] if ne_od else tod * P),
                            ):
                                itile = evi if colr[0] == 0 else odi
                                ha = (tcnt + 1) // 2
                                for lo, hi in ((0, ha), (ha, tcnt)):
                                    if lo >= hi:
                                        continue
                                    cnt = min(nreal, hi * P) - lo * P
                                    cnt = max(16, -(-cnt // 16) * 16)
                                    cnt = min(cnt, (hi - lo) * P)
                                    nc.gpsimd.dma_gather(
                                        out_ap=pool_t[:, lo:hi, :],
                                        in_ap=pairs[:, colr[0]:colr[1]],
                                        idxs_ap=itile[:, (lo) * 8:hi * 8],
                                        num_idxs=(hi - lo) * P,
                                        num_idxs_reg=cnt, elem_size=HC,
                                        elem_step=2 * HC, single_packet=False,
                                        queue_num=(4 * j + qn) % 4)
                                    qn += 1

                        msgden = eps.tile([P, HC + 4], dt.float32, tag="msgden",
                                          bufs=1)
                        groups = []
                        if layer == 1:
                            # no parity constraint: span the whole chunk
                            t0 = 0
                            while t0 < tj:
                                gb = min(GB, tj - t0)
                                groups.append((0, t0, gb, 0))
                                t0 += gb
                        else:
                            for base, tcnt, parity in ((0, tev, 0),
                                                       (tev, tod, 1)):
                                t0 = 0
                                while t0 < tcnt:
                                    gb = min(GB, tcnt - t0)
                                    groups.append((base, t0, gb, parity))
                                    t0 += gb

                        first = True
                        ngroups = len(groups)
                        for gi, (base, t0, gb, parity) in enumerate(groups):
                            glob0 = base + t0  # tile index within chunk
                            if layer == 1:
                                pool, poff = xjt, glob0
                            else:
                                pool = xj_ev if parity == 0 else xj_od
                                poff = t0

                            # --- z computed transposed: [hc_half, e] ---
                            # each half's accumulation group fully closes
                            # before the next half's start=True (PSUM
                            # has_written is bank-granular).
                            zt = gps.tile([P, 2, GB * P], dt.float32, tag="zt")
                            for hh in range(2):
                                nc.tensor.matmul(
                                    out=zt[:, hh, 0:gb * P],
                                    lhsT=hck[:, hh * P:(hh + 1) * P],
                                    rhs=sT[:, glob0 * P:(glob0 + gb) * P],
                                    start=True, stop=False)
                                for i in range(gb):
                                    nc.tensor.matmul(
                                        out=zt[:, hh, i * P:(i + 1) * P],
                                        lhsT=pool[:, poff + i,
                                                  hh * P:(hh + 1) * P],
                                        rhs=ident[:],
                                        start=False,
                                        stop=(i == gb - 1))

                            s_bT = work.tile([P, 2, GB * P], dt.bfloat16,
                                             tag="s_bT")
                            nc.scalar.activation(
                                s_bT[:, :, 0:gb * P], zt[:, :, 0:gb * P],
                                AF.Prelu, alpha=SLOPE)

                            # --- alpha[e, h] on PE ---
                            alph = gps.tile([P, GB, H], dt.float32, tag="alph",
                                            bufs=1)
                            for i in range(gb):
                                for hh in range(2):
                                    nc.tensor.matmul(
                                        out=alph[:, i, :],
                                        lhsT=s_bT[:, hh, i * P:(i + 1) * P],
                                        rhs=att_tile[:, hh, :],
                                        start=(hh == 0), stop=(hh == 1))

                            msg = work.tile([P, GB, HC + 4], dt.bfloat16,
                                            tag="msg")
                            nc.scalar.activation(
                                msg[:, 0:gb, HC:HC + 4],
                                alph[:, 0:gb, :], AF.Exp)
                            nc.vector.tensor_tensor(
                                out=msg[:, 0:gb, 0:HC].rearrange(
                                    "p a (h c) -> p a h c", h=H),
                                in0=pool[:, poff:poff + gb, :].rearrange(
                                    "p a (h c) -> p a h c", h=H),
                                in1=msg[:, 0:gb, HC:HC + 4].to_broadcast(
                                    [P, gb, H, C64]),
                                op=ALU.mult)
                            for i in range(gb):
                                last = (gi == ngroups - 1) and (i == gb - 1)
                                nc.tensor.matmul(
                                    out=msgden[:],
                                    lhsT=sen[:, (glob0 + i) * P:
                                             (glob0 + i + 1) * P],
                                    rhs=msg[:, i, :], start=first, stop=last)
                                first = False

                        # ---------------- epilogue ----------------
                        den = work.tile([P, H], dt.float32, tag="den")
                        nc.vector.tensor_scalar(
                            out=den[:], in0=msgden[:, HC:HC + 4], scalar1=1e-20,
                            scalar2=None, op0=ALU.max)
                        rden = work.tile([P, H], dt.float32, tag="rden")
                        nc.vector.reciprocal(rden[:], den[:])
                        onrm = work.tile([P, HC], dt.float32, tag="onrm")
                        nc.vector.tensor_tensor(
                            out=onrm[:].rearrange("p (h c) -> p h c", h=H),
                            in0=msgden[:, 0:HC].rearrange(
                                "p (h c) -> p h c", h=H),
                            in1=rden[:].to_broadcast([P, H, C64]),
                            op=ALU.mult)
                        orl = work.tile([P, HC], dt.bfloat16, tag="orl")
                        nc.scalar.activation(orl[:], onrm[:], AF.Relu)

                        trs = []
                        trpt = sps.tile([P, 2 * P], dt.bfloat16, tag="trp")
                        for half in range(2):
                            trp = trpt[:, half * P:(half + 1) * P]
                            nc.tensor.transpose(
                                out=trp, in_=orl[:, half * P:(half + 1) * P],
                                identity=ident[:])
                            tr = work.tile([P, P], dt.bfloat16, tag=f"trs{half}")
                            nc.vector.tensor_scalar(
                                out=tr[:], in0=trp, scalar1=0.0, scalar2=None,
                                op0=ALU.add)
                            trs.append(tr)

                        if layer == 1:
                            # inline phase B: h2 = relu1 @ W2 + b2
                            h2ps = sps.tile([P, HC], dt.float32, tag="pmm")
                            nc.tensor.matmul(out=h2ps[:], lhsT=trs[0][:],
                                             rhs=wtile["w2lo"][:],
                                             start=True, stop=False)
                            nc.tensor.matmul(out=h2ps[:], lhsT=trs[1][:],
                                             rhs=wtile["w2hi"][:],
                                             start=False, stop=not has_b2)
                            if has_b2:
                                nc.tensor.matmul(out=h2ps[:], lhsT=ones_row[:],
                                                 rhs=bias_t["b2"][:],
                                                 start=False, stop=True)
                            h2b = work.tile([P, HC], dt.bfloat16, tag="h2b")
                            nc.vector.tensor_scalar(
                                out=h2b[:], in0=h2ps[:], scalar1=0.0,
                                scalar2=None, op0=ALU.add)
                            nc.sync.dma_start(
                                out=h2in[k][(j - j0) * P:(j - j0 + 1) * P, :],
                                in_=h2b[:])
                            if debug_h2:
                                nc.sync.dma_start(
                                    out=h2_dbg[j * P:(j + 1) * P, :],
                                    in_=orl[:])
                            if j - j0 == sk - 1:
                                nc.gpsimd.collective_compute(
                                    "AllGather", mybir.AluOpType.bypass,
                                    replica_groups=rg,
                                    ins=[h2in[k].ap().opt()],
                                    outs=[h2_full[
                                        SLICE_STARTS[k] * NCORES * P:
                                        (SLICE_STARTS[k] + SLICE_SIZES[k])
                                        * NCORES * P, :].opt()])
                        else:
                            # inline phase C: y = (relu2 @ W3 + b3) @ W4 + b4
                            pmm = sps.tile([P, HC], dt.float32, tag="pmm")
                            ps3 = pmm[:, 0:OUT]
                            nc.tensor.matmul(out=ps3, lhsT=trs[0][:],
                                             rhs=wtile["w3lo"][:],
                                             start=True, stop=False)
                            nc.tensor.matmul(out=ps3, lhsT=trs[1][:],
                                             rhs=wtile["w3hi"][:],
                                             start=False, stop=not has_b34)
                            if has_b34:
                                nc.tensor.matmul(out=ps3, lhsT=ones_row[:],
                                                 rhs=bias_t["b3"][:],
                                                 start=False, stop=True)
                            h3 = work.tile([P, OUT], dt.bfloat16, tag="h3")
                            nc.vector.tensor_scalar(
                                out=h3[:], in0=ps3, scalar1=0.0, scalar2=None,
                                op0=ALU.add)
                            h3tp = trpt[0:OUT, 0:P]
                            nc.tensor.transpose(out=h3tp, in_=h3[:],
                                                identity=ident[:])
                            h3t = work.tile([OUT, P], dt.bfloat16, tag="h3t")
                            nc.vector.tensor_scalar(
                                out=h3t[:], in0=h3tp, scalar1=0.0, scalar2=None,
                                op0=ALU.add)
                            ps4 = pmm[:, OUT:2 * OUT]
                            nc.tensor.matmul(out=ps4, lhsT=h3t[:],
                                             rhs=wtile["w4"][:],
                                             start=True, stop=not has_b34)
                            if has_b34:
                                nc.tensor.matmul(out=ps4, lhsT=ones_row[:],
                                                 rhs=bias_t["b4"][:],
                                                 start=False, stop=True)
                            yt = work.tile([P, OUT], dt.float32, tag="yt")
                            nc.vector.tensor_scalar(
                                out=yt[:], in0=ps4, scalar1=0.0, scalar2=None,
                                op0=ALU.add)
                            nc.sync.dma_start(
                                out=y_shard[j * P:(j + 1) * P, :], in_=yt[:])

            edge_layer(1)
            edge_layer(2)

    nc.compile()
    return nc


# ----------------------------------------------------------------- kernel()

_CACHE = {}


def kernel(**inputs):
    from concourse.bass_utils import run_bass_kernel_spmd

    in_maps, perms, meta = prepare(inputs)
    key = tuple(sorted((k, tuple(v) if isinstance(v, tuple) else v)
                       for k, v in meta.items()))
    if key not in _CACHE:
        _CACHE[key] = build(**meta)
    nc = _CACHE[key]
    res = run_bass_kernel_spmd(nc, in_maps, core_ids=list(range(NCORES)))
    out = np.zeros((N, OUT), np.float32)
    for c in range(NCORES):
        ys = res.results[c]["y_shard"]
        valid = perms[c] >= 0
        out[perms[c][valid] + c * NPC] = ys[valid]
    return out


if __name__ == "__main__":
    import jax
    import reference
    cpu = jax.devices("cpu")[0]
    with jax.default_device(cpu):
        inputs = {k: np.asarray(v) for k, v in reference.setup_inputs().items()}
        exp = np.asarray(reference.reference(**inputs))
    got = kernel(**inputs)
    rel = np.linalg.norm(got - exp) / np.linalg.norm(exp)
    print("Relative error:", rel)
